# revision 1
# baseline (speedup 1.0000x reference)
"""Trainium2 Bass kernel for nn_AttentionalGNN (self-contained).

  xs/xt = standardize(p_src/p_tar).T ; ds/dt = mlp_dis(standardize(dis).T)
  delta0 = attn(xs, xt, xt); delta1 = attn(xt, xs, xs)
  ps = delta0*xt; pt = delta1*xs
  delta0' = attn(ds, dt, ps); delta1' = attn(dt, ds, pt)
  out_s = xs + mlp(cat(xs, delta0')); out_t likewise
  return ||mean_n(out_s) - mean_n(out_t)||^2

8-core SPMD: scale stats replicated; queries sharded 512/core for all four
attention calls (keys/values replicated); mlp_dis sharded over N with BN-stat
AllReduce + AllGather; round-1 deltas AllGathered (bf16) for the gating
products; final MLP sharded over N with AllReduced BN stats; final column
means AllReduced; the scalar is computed on every core.

Attention uses transposed scores (keys on partitions, queries on free) so no
transposes are needed anywhere: scoresT = K_h^T Q_h via one K=64 matmul per
key m-tile; exp on ScalarE (scale=1/8, no max subtraction - scores are O(10));
softmax denominator comes from a ones column prepended to V^T inside the PV
matmul (out partition 0 = denom, 1..64 = data); per-head normalization happens
at PSUM evacuation with a PE-broadcast reciprocal; the merge accumulates per
head with K=64 matmuls so everything stays partition-base-aligned. Head
channels are permuted host-side (d*4+h -> h*64+d) so head slices are
contiguous; merge weight columns permuted to match; V bias folded into the
merge bias (bm_eff = am_b + am_w @ av_b).
"""

import numpy as np
import ml_dtypes

D, H, HD, S, N, EPS = 256, 4, 64, 128, 4096, 1e-5
NC = 8
NQ = N // NC            # 512 queries per core
MT = N // 128           # 32 key m-tiles
HB = HD + 1             # per-head V^T block: [ones | V] = 65 cols

_CACHE = {}


def _build_program(dbg=False, single=False):
    """single=True: replace collectives with same-size local DMA copies so the
    program is single-core simulatable (TimelineSim) - timing proxy only."""
    import contextlib
    import concourse.bass as bass
    import concourse.bacc as bacc
    import concourse.tile as tile
    import concourse.mybir as mybir

    FP32 = mybir.dt.float32
    BF16 = mybir.dt.bfloat16
    AF = mybir.ActivationFunctionType
    ALU = mybir.AluOpType
    AX = mybir.AxisListType

    nc = bacc.Bacc(
        "TRN2",
        target_bir_lowering=False,
        debug=False,
        enable_asserts=False,
        num_devices=NC,
    )

    def din(name, shape, dt):
        return nc.dram_tensor(name, shape, dt, kind="ExternalInput").ap()

    psT = din("psT", [D, N], FP32)
    ptT = din("ptT", [D, N], FP32)
    dsT = din("dsT", [S, N], FP32)
    dtT = din("dtT", [S, N], FP32)
    ops = din("ops", [D, NQ], FP32)
    opt_ = din("opt", [D, NQ], FP32)
    ods = din("ods", [S, NQ], FP32)
    odt = din("odt", [S, NQ], FP32)
    wqT = din("wqT", [D, D], BF16)
    wkT = din("wkT", [D, D], BF16)
    wvT = din("wvT", [D, D], BF16)
    wmT = din("wmT", [D, D], BF16)
    bqv = din("bq", [D, 1], FP32)
    bkv = din("bk", [D, 1], FP32)
    bmv = din("bm", [D, 1], FP32)
    m1T = din("m1T", [2 * D, 2 * D], BF16)
    m1b = din("m1b", [2 * D, 1], FP32)
    m1g = din("m1g", [2 * D, 1], FP32)
    m1be = din("m1be", [2 * D, 1], FP32)
    m2T = din("m2T", [2 * D, D], BF16)
    m2b = din("m2b", [D, 1], FP32)
    d1T = din("d1T", [S, D], BF16)
    d1b = din("d1b", [D, 1], FP32)
    d1g = din("d1g", [D, 1], FP32)
    d1be = din("d1be", [D, 1], FP32)
    d2T = din("d2T", [D, D], BF16)
    d2b = din("d2b", [D, 1], FP32)
    out_dram = nc.dram_tensor("out", [1, 1], FP32, kind="ExternalOutput").ap()

    RG = [list(range(NC))]
    _dbg_done = set()

    def mkdbg(dma_fn, name, src_ap, shape, dt):
        if not dbg or name in _dbg_done:
            return
        _dbg_done.add(name)
        d = nc.dram_tensor(name, list(shape), dt, kind="ExternalOutput").ap()
        dma_fn(d[tuple(slice(None) for _ in shape)], src_ap)

    with tile.TileContext(nc) as tc:
        st = contextlib.ExitStack()
        PA = st.enter_context(tc.tile_pool(name="persistA", bufs=1))
        Ppr = st.enter_context(tc.tile_pool(name="probs", bufs=4))
        Psc = st.enter_context(
            tc.tile_pool(name="psum_sc", bufs=3, space=bass.MemorySpace.PSUM))
        Pout = st.enter_context(
            tc.tile_pool(name="psum_out", bufs=1, space=bass.MemorySpace.PSUM))
        Pbc = st.enter_context(
            tc.tile_pool(name="psum_bc", bufs=1, space=bass.MemorySpace.PSUM))
        Dram = st.enter_context(tc.tile_pool(name="dram", bufs=1, space="DRAM"))

        def pa(name, shape, dt, tag=None):
            return PA.tile(shape, dt, name=name, tag=tag or name)

        # --- persistA: needed from preprocessing onward ---
        xs_bf = pa("xs_bf", [128, 2, 8, 512], BF16)
        xt_bf = pa("xt_bf", [128, 2, 8, 512], BF16)
        xs_own = pa("xs_own", [128, 2, 512], FP32)
        xt_own = pa("xt_own", [128, 2, 512], FP32)
        xs_own_bf = pa("xs_own_bf", [128, 2, 512], BF16)
        xt_own_bf = pa("xt_own_bf", [128, 2, 512], BF16)
        ds_own_bf = pa("ds_own_bf", [128, 2, 512], BF16)
        dt_own_bf = pa("dt_own_bf", [128, 2, 512], BF16)
        x1d_s = pa("x1d_s", [128, 2, 512], FP32)
        x1d_t = pa("x1d_t", [128, 2, 512], FP32)
        sc_ds = pa("sc_ds", [128, 512], BF16)
        sc_dt = pa("sc_dt", [128, 512], BF16)
        r_d = pa("r_d", [128, 2, 512], BF16)
        Qb = pa("Qb", [128, 2, 512], BF16)
        An = pa("An", [64, 4, 512], BF16)       # attn out per head (raw->normed)
        d_bf = pa("d_bf", [128, 2, 512], BF16)  # round-1 delta0 own
        d1own_bf = pa("d1own_bf", [128, 2, 512], BF16)
        dp0_f = pa("dp0_f", [128, 2, 512], FP32)
        dp1_f = pa("dp1_f", [128, 2, 512], FP32)
        dp0_bf = pa("dp0_bf", [128, 2, 512], BF16)
        dp1_bf = pa("dp1_bf", [128, 2, 512], BF16)
        ones = pa("ones", [128, 64], FP32)
        wq_s = pa("wq_s", [128, 2, 256], BF16)
        wk_s = pa("wk_s", [128, 2, 256], BF16)
        wv_s = pa("wv_s", [128, 2, 256], BF16)
        wm_r = pa("wm_r", [64, 4, 256], BF16)   # head-major merge weights
        m1_s = pa("m1_s", [128, 4, 512], BF16)
        m2_s = pa("m2_s", [128, 4, 256], BF16)
        d1_s = pa("d1_s", [128, 256], BF16)
        d2_s = pa("d2_s", [128, 2, 256], BF16)
        bq_s = pa("bq_s", [128, 2, 1], FP32)
        bk_s = pa("bk_s", [128, 2, 1], FP32)
        bm_s = pa("bm_s", [128, 2, 1], FP32)
        m1b_s = pa("m1b_s", [128, 4, 1], FP32)
        m1g_s = pa("m1g_s", [128, 4, 1], FP32)
        m1be_s = pa("m1be_s", [128, 4, 1], FP32)
        m2b_s = pa("m2b_s", [128, 2, 1], FP32)
        d1b_s = pa("d1b_s", [128, 2, 1], FP32)
        d1g_s = pa("d1g_s", [128, 2, 1], FP32)
        d1be_s = pa("d1be_s", [128, 2, 1], FP32)
        d2b_s = pa("d2b_s", [128, 2, 1], FP32)
        bns = pa("bns", [128, 8, 6], FP32)
        par = pa("par", [128, 2, 4], FP32)
        tot = pa("tot", [128, 2, 4], FP32)
        parf = pa("parf", [128, 4, 4], FP32)
        totf = pa("totf", [128, 4, 4], FP32)
        pars = pa("pars", [128, 2, 2], FP32)
        tots = pa("tots", [128, 2, 2], FP32)
        dlt = pa("dlt", [128, 2, 1], FP32)
        dsq = pa("dsq", [128, 2, 1], FP32)
        res = pa("res", [1, 1], FP32)
        sv = pa("sv", [128, 200], FP32)   # scalar scratch, allocator below

        _svc = [0]

        def scol(n=1):
            b = _svc[0]
            _svc[0] += n
            assert _svc[0] <= 200
            return [sv[:, b + i:b + i + 1] for i in range(n)]

        dma = nc.sync.dma_start
        shr = "Local" if single else "Shared"

        def coll(kind, op, in_t, out_t, in_rows):
            if not single:
                nc.gpsimd.collective_compute(kind, op, replica_groups=RG,
                                             ins=[in_t.opt()],
                                             outs=[out_t.opt()])
            elif kind == "AllGather":
                for r_ in range(NC):
                    dma(out_t[r_ * in_rows:(r_ + 1) * in_rows, :], in_t[:, :])
            else:
                dma(out_t[:, :], in_t[:, :])

        # ---------------- weights / biases ----------------
        for g in range(2):
            r = slice(g * 128, (g + 1) * 128)
            dma(wq_s[:, g, :], wqT[r, :])
            dma(wk_s[:, g, :], wkT[r, :])
            dma(wv_s[:, g, :], wvT[r, :])
            dma(d2_s[:, g, :], d2T[r, :])
            dma(bq_s[:, g, :], bqv[r, :])
            dma(bk_s[:, g, :], bkv[r, :])
            dma(bm_s[:, g, :], bmv[r, :])
            dma(m2b_s[:, g, :], m2b[r, :])
            dma(d1b_s[:, g, :], d1b[r, :])
            dma(d1g_s[:, g, :], d1g[r, :])
            dma(d1be_s[:, g, :], d1be[r, :])
            dma(d2b_s[:, g, :], d2b[r, :])
        dma(d1_s[:, :], d1T[:, :])
        for h in range(H):
            dma(wm_r[:, h, :], wmT[h * 64:(h + 1) * 64, :])
        for g in range(4):
            r = slice(g * 128, (g + 1) * 128)
            dma(m1_s[:, g, :], m1T[r, :])
            dma(m2_s[:, g, :], m2T[r, :])
            dma(m1b_s[:, g, :], m1b[r, :])
            dma(m1g_s[:, g, :], m1g[r, :])
            dma(m1be_s[:, g, :], m1be[r, :])
        nc.gpsimd.memset(ones[:, :], 1.0)

        def inv_std(var_ap, eps):
            t, s0, r0, s1, inv = scol(5)
            nc.vector.tensor_scalar_add(t, var_ap, float(eps))
            nc.scalar.activation(s0, t, AF.Sqrt)
            nc.vector.reciprocal(r0, s0)
            nc.vector.tensor_mul(r0, t, r0)
            nc.vector.tensor_add(r0, r0, s0)
            nc.vector.tensor_scalar_mul(s1, r0, 0.5)
            nc.vector.reciprocal(inv, s1)
            return inv

        # ------------- standardize p_src/p_tar (per row-group stream) -------------
        with tc.tile_pool(name="pin", bufs=2) as Pin:
            for nm, srcT, ownT, dst_bf, own_f32, own_b16 in (
                ("s", psT, ops, xs_bf, xs_own, xs_own_bf),
                ("t", ptT, opt_, xt_bf, xt_own, xt_own_bf),
            ):
                for g in range(2):
                    pbuf = Pin.tile([128, 8, 512], FP32, tag="pbuf",
                                    name=f"pbuf_{nm}{g}")
                    obuf = Pin.tile([128, 512], FP32, tag="obuf",
                                    name=f"obuf_{nm}{g}")
                    r = slice(g * 128, (g + 1) * 128)
                    dma(pbuf[:, :, :], srcT[r, :].rearrange("p (c f) -> p c f", f=512))
                    dma(obuf[:, :], ownT[r, :])
                    for c in range(8):
                        nc.vector.bn_stats(bns[:, c, :], pbuf[:, c, :])
                    ag2 = pa(f"ag_{nm}{g}", [128, 2], FP32)
                    nc.vector.bn_aggr(ag2[:, :], bns[:, :, :])
                    inv = inv_std(ag2[:, 1:2], 0.0)
                    (nb,) = scol(1)
                    nc.vector.tensor_mul(nb, ag2[:, 0:1], inv)
                    nc.vector.tensor_scalar_mul(nb, nb, -1.0)
                    nc.scalar.activation(dst_bf[:, g, :, :], pbuf[:, :, :],
                                         AF.Identity, bias=nb, scale=inv)
                    nc.scalar.activation(own_f32[:, g, :], obuf[:, :],
                                         AF.Identity, bias=nb, scale=inv)
                    nc.vector.tensor_copy(own_b16[:, g, :], own_f32[:, g, :])

            # ------------- dis stats + own shard scale + d1 conv -------------
            for nm, srcT, ownT, scdst in (
                ("ds", dsT, ods, sc_ds),
                ("dt", dtT, odt, sc_dt),
            ):
                dbuf = Pin.tile([128, 8, 512], FP32, tag="pbuf", name=f"dbuf_{nm}")
                obuf = Pin.tile([128, 512], FP32, tag="obuf", name=f"obuf_{nm}")
                dma(dbuf[:, :, :], srcT[:, :].rearrange("p (c f) -> p c f", f=512))
                dma(obuf[:, :], ownT[:, :])
                for c in range(8):
                    nc.vector.bn_stats(bns[:, c, :], dbuf[:, c, :])
                ag2 = pa(f"ag_{nm}", [128, 2], FP32)
                nc.vector.bn_aggr(ag2[:, :], bns[:, :, :])
                inv = inv_std(ag2[:, 1:2], 0.0)
                (nb,) = scol(1)
                nc.vector.tensor_mul(nb, ag2[:, 0:1], inv)
                nc.vector.tensor_scalar_mul(nb, nb, -1.0)
                nc.scalar.activation(scdst[:, :], obuf[:, :],
                                     AF.Identity, bias=nb, scale=inv)

        for src, dst in ((sc_ds, x1d_s), (sc_dt, x1d_t)):
            for og in range(2):
                mp = Psc.tile([128, 2, 512], FP32, tag="sc", name="mp_d1")
                nc.tensor.matmul(mp[:, 0, :], d1_s[:, og * 128:(og + 1) * 128],
                                 src[:, :], start=True, stop=True)
                nc.vector.tensor_scalar_add(dst[:, og, :], mp[:, 0, :],
                                            d1b_s[:, og, :])
        # partial BN stats for both d1 outputs -> one AllReduce
        sqd = pa("sqd", [128, 2, 512], FP32)
        for i, x1 in enumerate((x1d_s, x1d_t)):
            nc.vector.reduce_sum(par[:, :, 2 * i], x1[:, :, :], axis=AX.X)
            nc.scalar.activation(sqd[:, :, :], x1[:, :, :], AF.Square)
            nc.vector.reduce_sum(par[:, :, 2 * i + 1], sqd[:, :, :], axis=AX.X)
        ar_in = Dram.tile([D, 4], FP32, name="ar_in")
        ar_out = Dram.tile([D, 4], FP32, name="ar_out", addr_space=shr)
        for g in range(2):
            dma(ar_in[g * 128:(g + 1) * 128, :], par[:, g, :])
        coll("AllReduce", ALU.add, ar_in, ar_out, D)

        # --- persistB: attention-era tensors ---
        PB = st.enter_context(tc.tile_pool(name="persistB", bufs=1))

        def pb(name, shape, dt, tag=None):
            return PB.tile(shape, dt, name=name, tag=tag or name)

        VT = pb("VT", [128, MT, H * HB], BF16)
        Kb = pb("Kb", [128, 2, 8, 512], BF16)
        ds_bf = pb("ds_bf", [128, 2, 8, 512], BF16)
        dt_bf = pb("dt_bf", [128, 2, 8, 512], BF16)
        d0f = pb("d0f", [128, 2, 8, 512], BF16)
        for h in range(H):
            nc.gpsimd.memset(VT[:, :, h * HB + HD], 1.0)

        def attention(tag, q_own_bf, k_src, v_src, merge_f32, merge_b16):
            # Q projection (+bias)
            qp = Psc.tile([128, 2, 512], FP32, tag="sc", name=f"qp_{tag}")
            for og in range(2):
                for cg in range(2):
                    nc.tensor.matmul(qp[:, og, :],
                                     wq_s[:, cg, og * 128:(og + 1) * 128],
                                     q_own_bf[:, cg, :],
                                     start=(cg == 0), stop=(cg == 1))
            for og in range(2):
                nc.scalar.activation(Qb[:, og, :], qp[:, og, :], AF.Identity,
                                     bias=bq_s[:, og, :])
            # K projection (+bias), full N
            for og in range(2):
                for c in range(8):
                    kp = Psc.tile([128, 2, 512], FP32, tag="sc", name=f"kp_{tag}")
                    for cg in range(2):
                        nc.tensor.matmul(kp[:, 0, :],
                                         wk_s[:, cg, og * 128:(og + 1) * 128],
                                         k_src[:, cg, c, :],
                                         start=(cg == 0), stop=(cg == 1))
                    nc.vector.tensor_scalar_add(Kb[:, og, c, :], kp[:, 0, :],
                                                bk_s[:, og, :])
            # V^T projection (keys on partitions), no bias (folded into bm)
            for m in range(MT):
                c, f0 = divmod(m * 128, 512)
                vp = Psc.tile([128, 2, 512], FP32, tag="sc", name=f"vp_{tag}")
                for cg in range(2):
                    nc.tensor.matmul(vp[:, 0, 0:256],
                                     v_src[:, cg, c, f0:f0 + 128],
                                     wv_s[:, cg, :],
                                     start=(cg == 0), stop=(cg == 1))
                nc.vector.tensor_copy(
                    VT[:, m, :].rearrange("p (h c) -> p h c", c=HB)[:, :, 0:HD],
                    vp[:, 0, 0:256].rearrange("p (h c) -> p h c", c=HD))
            # streaming attention per head, PV pipelined one group behind
            recs = []
            for h in range(H):
                hg, hp = h // 2, (h % 2) * 64
                op = Pout.tile([65, 512], FP32, tag="out", name=f"op_{tag}{h}")
                prev = None
                for g in range(16):
                    sc = Psc.tile([128, 2, 512], FP32, tag="sc", name=f"sc_{tag}")
                    for j in range(2):
                        m = g * 2 + j
                        c, f0 = divmod(m * 128, 512)
                        nc.tensor.matmul(sc[:, j, :],
                                         Kb[hp:hp + 64, hg, c, f0:f0 + 128],
                                         Qb[hp:hp + 64, hg, :],
                                         start=True, stop=True)
                    pr = Ppr.tile([128, 2, 512], BF16, tag="pr", name=f"pr_{tag}",
                                  bufs=3)
                    nc.scalar.activation(pr[:, :, :], sc[:, :, :], AF.Exp,
                                         scale=0.125)
                    if prev is not None:
                        for j in range(2):
                            m = prev[0] * 2 + j
                            nc.tensor.matmul(
                                op[:, :], VT[:, m, h * HB:(h + 1) * HB],
                                prev[1][:, j, :], start=(m == 0), stop=False)
                    prev = (g, pr)
                for j in range(2):
                    m = prev[0] * 2 + j
                    nc.tensor.matmul(op[:, :], VT[:, m, h * HB:(h + 1) * HB],
                                     prev[1][:, j, :], start=False,
                                     stop=(m == MT - 1))
                # evacuate raw numerator (bf16); denominator: psum row 64 ->
                # sbuf row 64 (ScalarE) -> partition 0 (DMA) -> reciprocal at
                # base 0 (custom DVE op misbehaves at base 64). The PE
                # broadcast + normalize are deferred past the head loop so the
                # PE never stalls on this chain.
                nc.vector.tensor_copy(An[:, h, :], op[0:64, :])
                dnm = Ppr.tile([65, 512], FP32, tag="dnm", bufs=4,
                               name=f"dnm_{tag}{h}")
                nc.scalar.activation(dnm[64:65, :], op[64:65, :], AF.Copy)
                dma(dnm[0:1, :], dnm[64:65, :])
                rc = Ppr.tile([1, 512], FP32, tag="rc", bufs=4,
                              name=f"rc_{tag}{h}")
                nc.vector.reciprocal_approx_fast(rc[0:1, :], dnm[0:1, :])
                recs.append(rc)
            for h in range(H):
                bc = Pbc.tile([64, 512], FP32, tag="bc", name=f"bc_{tag}{h}")
                nc.tensor.matmul(bc[:, :], ones[0:1, 0:64], recs[h][0:1, :],
                                 start=True, stop=True)
                nc.vector.tensor_mul(An[:, h, :], An[:, h, :], bc[:, :])
            # merge: accumulate per head (K=64), + bm_eff at evacuation
            mg = Psc.tile([128, 2, 512], FP32, tag="sc", name=f"mg_{tag}")
            for og in range(2):
                for h in range(H):
                    nc.tensor.matmul(mg[:, og, :],
                                     wm_r[:, h, og * 128:(og + 1) * 128],
                                     An[:, h, :],
                                     start=(h == 0), stop=(h == 3))
            for og in range(2):
                if merge_f32 is not None:
                    nc.scalar.activation(merge_f32[:, og, :], mg[:, og, :],
                                         AF.Identity, bias=bm_s[:, og, :])
                    if merge_b16 is not None:
                        nc.vector.tensor_copy(merge_b16[:, og, :],
                                              merge_f32[:, og, :])
                else:
                    nc.scalar.activation(merge_b16[:, og, :], mg[:, og, :],
                                         AF.Identity, bias=bm_s[:, og, :])

        # ---------------- round 1a (dis AllReduce completes underneath) ----------
        ag_in = Dram.tile([2 * D, NQ], BF16, name="ag_in")
        ag_out = Dram.tile([NC * 2 * D, NQ], BF16, name="ag_out",
                           addr_space=shr)
        attention("r1a", xs_own_bf, xt_bf, xt_bf, None, d_bf)
        mkdbg(dma, "dbg_xs_own", xs_own[:, :, :], (128, 2, 512), FP32)
        mkdbg(dma, "dbg_xs_bf", xs_bf[:, :, :, :], (128, 2, 8, 512), BF16)
        mkdbg(dma, "dbg_qb", Qb[:, :, :], (128, 2, 512), BF16)
        mkdbg(dma, "dbg_kb", Kb[:, :, :, :], (128, 2, 8, 512), BF16)
        mkdbg(dma, "dbg_vt", VT[:, :, :], (128, MT, H * HB), BF16)
        mkdbg(dma, "dbg_an", An[:, :, :], (64, 4, 512), BF16)
        mkdbg(dma, "dbg_dbf", d_bf[:, :, :], (128, 2, 512), BF16)
        for g in range(2):
            dma(ag_in[g * 128:(g + 1) * 128, :], d_bf[:, g, :])

        # ---- dis BN apply + relu + d2 (own shard) + AllGather ds/dt ----
        for g in range(2):
            dma(tot[:, g, :], ar_out[g * 128:(g + 1) * 128, :])
        for i, (x1, dst) in enumerate(((x1d_s, ds_own_bf), (x1d_t, dt_own_bf))):
            for og in range(2):
                mu, va, msq, a_, b_ = scol(5)
                nc.vector.tensor_scalar_mul(mu, tot[:, og, 2 * i:2 * i + 1],
                                            1.0 / N)
                nc.vector.tensor_scalar_mul(va, tot[:, og, 2 * i + 1:2 * i + 2],
                                            1.0 / N)
                nc.vector.tensor_mul(msq, mu, mu)
                nc.vector.tensor_sub(va, va, msq)
                inv = inv_std(va, EPS)
                nc.vector.tensor_mul(a_, d1g_s[:, og, :], inv)
                nc.vector.tensor_mul(b_, mu, a_)
                nc.vector.tensor_scalar_mul(b_, b_, -1.0)
                nc.vector.tensor_add(b_, b_, d1be_s[:, og, :])
                nc.scalar.activation(r_d[:, og, :], x1[:, og, :], AF.Relu,
                                     bias=b_, scale=a_)
            for og in range(2):
                mp = Psc.tile([128, 2, 512], FP32, tag="sc", name="mp_d2")
                for cg in range(2):
                    nc.tensor.matmul(mp[:, 0, :],
                                     d2_s[:, cg, og * 128:(og + 1) * 128],
                                     r_d[:, cg, :], start=(cg == 0), stop=(cg == 1))
                nc.vector.tensor_scalar_add(dst[:, og, :], mp[:, 0, :],
                                            d2b_s[:, og, :])
        mkdbg(dma, "dbg_x1d", x1d_s[:, :, :], (128, 2, 512), FP32)
        mkdbg(dma, "dbg_tot", tot[:, :, :], (128, 2, 4), FP32)
        mkdbg(dma, "dbg_dsown", ds_own_bf[:, :, :], (128, 2, 512), BF16)
        agd_in = Dram.tile([2 * D, NQ], BF16, name="agd_in")
        agd_out = Dram.tile([NC * 2 * D, NQ], BF16, name="agd_out",
                            addr_space=shr)
        for g in range(2):
            dma(agd_in[g * 128:(g + 1) * 128, :], ds_own_bf[:, g, :])
            dma(agd_in[256 + g * 128:256 + (g + 1) * 128, :], dt_own_bf[:, g, :])
        coll("AllGather", ALU.bypass, agd_in, agd_out, 2 * D)

        # ---------------- round 1b (ds/dt AllGather completes underneath) --------
        attention("r1b", xt_own_bf, xs_bf, xs_bf, None, d1own_bf)
        for g in range(2):
            dma(ag_in[256 + g * 128:256 + (g + 1) * 128, :], d1own_bf[:, g, :])
        coll("AllGather", ALU.bypass, ag_in, ag_out, 2 * D)

        # gather ds/dt full
        for r in range(NC):
            for g in range(2):
                b0 = r * 2 * D
                dma(ds_bf[:, g, r, :], agd_out[b0 + g * 128:b0 + (g + 1) * 128, :])
                dma(dt_bf[:, g, r, :],
                    agd_out[b0 + 256 + g * 128:b0 + 256 + (g + 1) * 128, :])
        # gather deltas + gating: d0f = delta0*xt ; xt_bf <- delta1*xs (pt_tmp)
        for r in range(NC):
            b0 = r * 2 * D
            for g in range(2):
                dma(d0f[:, g, r, :], ag_out[b0 + g * 128:b0 + (g + 1) * 128, :])
        for g in range(2):
            nc.vector.tensor_mul(d0f[:, g, :, :], d0f[:, g, :, :],
                                 xt_bf[:, g, :, :])
        for r in range(NC):
            b0 = r * 2 * D
            for g in range(2):
                dma(xt_bf[:, g, r, :],
                    ag_out[b0 + 256 + g * 128:b0 + (g + 1) * 128 + 256, :])
        for g in range(2):
            nc.vector.tensor_mul(xt_bf[:, g, :, :], xt_bf[:, g, :, :],
                                 xs_bf[:, g, :, :])

        mkdbg(dma, "dbg_dsbf", ds_bf[:, :, :, :], (128, 2, 8, 512), BF16)
        mkdbg(dma, "dbg_d0f", d0f[:, :, :, :], (128, 2, 8, 512), BF16)
        mkdbg(dma, "dbg_pt", xt_bf[:, :, :, :], (128, 2, 8, 512), BF16)

        # ---------------- round 2 ----------------
        attention("r2a", ds_own_bf, dt_bf, d0f, dp0_f, dp0_bf)
        mkdbg(dma, "dbg_dp0", dp0_f[:, :, :], (128, 2, 512), FP32)
        attention("r2b", dt_own_bf, ds_bf, xt_bf, dp1_f, dp1_bf)
        mkdbg(dma, "dbg_dp1", dp1_f[:, :, :], (128, 2, 512), FP32)

        # ---------------- final mlp (sharded) + MMD ----------------
        x1_s = PB.tile([128, 4, 512], FP32, name="x1_s", tag="d0f")
        x1_t = PB.tile([128, 4, 512], FP32, name="x1_t", tag="Kb")
        sq = PB.tile([128, 4, 512], FP32, name="sq", tag="VT")
        for i, (xo, dp, x1) in enumerate(((xs_own_bf, dp0_bf, x1_s),
                                          (xt_own_bf, dp1_bf, x1_t))):
            rhs = [xo[:, 0, :], xo[:, 1, :], dp[:, 0, :], dp[:, 1, :]]
            for og in range(4):
                mp = Psc.tile([128, 2, 512], FP32, tag="sc", name="mp_m1")
                for cg in range(4):
                    nc.tensor.matmul(mp[:, 0, :],
                                     m1_s[:, cg, og * 128:(og + 1) * 128],
                                     rhs[cg], start=(cg == 0), stop=(cg == 3))
                nc.vector.tensor_scalar_add(x1[:, og, :], mp[:, 0, :],
                                            m1b_s[:, og, :])
            nc.vector.reduce_sum(parf[:, :, 2 * i], x1[:, :, :], axis=AX.X)
            nc.scalar.activation(sq[:, :, :], x1[:, :, :], AF.Square)
            nc.vector.reduce_sum(parf[:, :, 2 * i + 1], sq[:, :, :], axis=AX.X)
        arf_in = Dram.tile([2 * D, 4], FP32, name="arf_in")
        arf_out = Dram.tile([2 * D, 4], FP32, name="arf_out", addr_space=shr)
        for g in range(4):
            dma(arf_in[g * 128:(g + 1) * 128, :], parf[:, g, :])
        coll("AllReduce", ALU.add, arf_in, arf_out, 2 * D)
        for g in range(4):
            dma(totf[:, g, :], arf_out[g * 128:(g + 1) * 128, :])

        mkdbg(dma, "dbg_x1s", x1_s[:, :, :], (128, 4, 512), FP32)
        mkdbg(dma, "dbg_totf", totf[:, :, :], (128, 4, 4), FP32)
        os_own = PB.tile([128, 2, 512], FP32, name="os_own", tag="ds_bf")
        ot_own = PB.tile([128, 2, 512], FP32, name="ot_own", tag="dt_bf")
        r_f = PB.tile([128, 4, 512], BF16, name="r_f", tag="VT")
        for i, (x1, xo, oo) in enumerate(((x1_s, xs_own, os_own),
                                          (x1_t, xt_own, ot_own))):
            for og in range(4):
                mu, va, msq, a_, b_ = scol(5)
                nc.vector.tensor_scalar_mul(mu, totf[:, og, 2 * i:2 * i + 1],
                                            1.0 / N)
                nc.vector.tensor_scalar_mul(va, totf[:, og, 2 * i + 1:2 * i + 2],
                                            1.0 / N)
                nc.vector.tensor_mul(msq, mu, mu)
                nc.vector.tensor_sub(va, va, msq)
                inv = inv_std(va, EPS)
                nc.vector.tensor_mul(a_, m1g_s[:, og, :], inv)
                nc.vector.tensor_mul(b_, mu, a_)
                nc.vector.tensor_scalar_mul(b_, b_, -1.0)
                nc.vector.tensor_add(b_, b_, m1be_s[:, og, :])
                nc.scalar.activation(r_f[:, og, :], x1[:, og, :], AF.Relu,
                                     bias=b_, scale=a_)
            for og in range(2):
                mp = Psc.tile([128, 2, 512], FP32, tag="sc", name="mp_m2")
                for cg in range(4):
                    nc.tensor.matmul(mp[:, 0, :],
                                     m2_s[:, cg, og * 128:(og + 1) * 128],
                                     r_f[:, cg, :], start=(cg == 0), stop=(cg == 3))
                nc.vector.scalar_tensor_tensor(
                    oo[:, og, :], mp[:, 0, :], m2b_s[:, og, :], xo[:, og, :],
                    op0=ALU.add, op1=ALU.add)

        mkdbg(dma, "dbg_os", os_own[:, :, :], (128, 2, 512), FP32)
        mkdbg(dma, "dbg_ot", ot_own[:, :, :], (128, 2, 512), FP32)
        nc.vector.reduce_sum(pars[:, :, 0], os_own[:, :, :], axis=AX.X)
        nc.vector.reduce_sum(pars[:, :, 1], ot_own[:, :, :], axis=AX.X)
        ars_in = Dram.tile([D, 2], FP32, name="ars_in")
        ars_out = Dram.tile([D, 2], FP32, name="ars_out", addr_space=shr)
        for g in range(2):
            dma(ars_in[g * 128:(g + 1) * 128, :], pars[:, g, :])
        coll("AllReduce", ALU.add, ars_in, ars_out, D)
        for g in range(2):
            dma(tots[:, g, :], ars_out[g * 128:(g + 1) * 128, :])
        mkdbg(dma, "dbg_tots", tots[:, :, :], (128, 2, 2), FP32)
        for g in range(2):
            nc.vector.tensor_sub(dlt[:, g, :], tots[:, g, 0:1], tots[:, g, 1:2])
        nc.vector.tensor_scalar_mul(dlt[:, :, :], dlt[:, :, :], 1.0 / N)
        nc.scalar.activation(dsq[:, :, :], dlt[:, :, :], AF.Square)
        dot = Pbc.tile([65, 512], FP32, tag="bc", name="dot")
        for g in range(2):
            nc.tensor.matmul(dot[0:1, 0:1], dsq[:, g, :], ones[:, 0:1],
                             start=(g == 0), stop=(g == 1))
        nc.vector.tensor_copy(res[:, :], dot[0:1, 0:1])
        dma(out_dram[:, :], res[:, :])

        st.close()

    nc.compile()
    return nc


# head permutation: new row i = h*64+d  <- old channel d*4+h
_PERM = np.array([d * H + h for h in range(H) for d in range(HD)])


def _prep_inputs(inputs):
    bf16 = ml_dtypes.bfloat16
    f32 = np.float32

    def C(x, dt=f32):
        return np.ascontiguousarray(np.asarray(x), dtype=dt)

    p_src = C(inputs["p_src"])[0]
    p_tar = C(inputs["p_tar"])[0]
    dis_src = C(inputs["dis_src"])[0]
    dis_tar = C(inputs["dis_tar"])[0]
    aq_w = C(inputs["aq_w"]); ak_w = C(inputs["ak_w"])
    av_w = C(inputs["av_w"]); am_w = C(inputs["am_w"])
    shared = {
        "psT": C(p_src.T), "ptT": C(p_tar.T),
        "dsT": C(dis_src.T), "dtT": C(dis_tar.T),
        "wqT": C(aq_w[_PERM, :].T, bf16),
        "wkT": C(ak_w[_PERM, :].T, bf16),
        "wvT": C(av_w[_PERM, :].T, bf16),
        "wmT": C(am_w[:, _PERM].T, bf16),
        "bq": C(inputs["aq_b"])[_PERM].reshape(D, 1).copy(),
        "bk": C(inputs["ak_b"])[_PERM].reshape(D, 1).copy(),
        "bm": (C(inputs["am_b"]) + am_w @ C(inputs["av_b"])).reshape(D, 1),
        "m1T": C(C(inputs["m1_w"]).T, bf16),
        "m1b": C(inputs["m1_b"]).reshape(2 * D, 1),
        "m1g": C(inputs["m1_g"]).reshape(2 * D, 1),
        "m1be": C(inputs["m1_be"]).reshape(2 * D, 1),
        "m2T": C(C(inputs["m2_w"]).T, bf16),
        "m2b": C(inputs["m2_b"]).reshape(D, 1),
        "d1T": C(C(inputs["d1_w"]).T, bf16),
        "d1b": C(inputs["d1_b"]).reshape(D, 1),
        "d1g": C(inputs["d1_g"]).reshape(D, 1),
        "d1be": C(inputs["d1_be"]).reshape(D, 1),
        "d2T": C(C(inputs["d2_w"]).T, bf16),
        "d2b": C(inputs["d2_b"]).reshape(D, 1),
    }
    in_maps = []
    for c in range(NC):
        sl = slice(c * NQ, (c + 1) * NQ)
        m = dict(shared)
        m["ops"] = C(p_src[sl, :].T)
        m["opt"] = C(p_tar[sl, :].T)
        m["ods"] = C(dis_src[sl, :].T)
        m["odt"] = C(dis_tar[sl, :].T)
        in_maps.append(m)
    return in_maps


def kernel(**inputs):
    from concourse.bass_utils import run_bass_kernel_spmd

    if "nc" not in _CACHE:
        _CACHE["nc"] = _build_program()
    nc = _CACHE["nc"]
    in_maps = _prep_inputs(inputs)
    res = run_bass_kernel_spmd(nc, in_maps, core_ids=list(range(NC)))
    return np.asarray(res.results[0]["out"], np.float32).reshape(())



# revision 32
# speedup vs baseline: 26.8861x; 26.8861x over previous
"""Trainium2 Bass kernel for nn_AttentionalGNN (self-contained).

  xs/xt = standardize(p_src/p_tar).T ; ds/dt = mlp_dis(standardize(dis).T)
  delta0 = attn(xs, xt, xt); delta1 = attn(xt, xs, xs)
  ps = delta0*xt; pt = delta1*xs
  delta0' = attn(ds, dt, ps); delta1' = attn(dt, ds, pt)
  out_s = xs + mlp(cat(xs, delta0')); out_t likewise
  return ||mean_n(out_s) - mean_n(out_t)||^2

8-core SPMD, queries sharded 512/core for all four attention calls
(keys/values replicated). Structural choices:

- All inputs packed into ONE bf16 [128, W] tensor + ONE tiny fp32 bias
  tensor per core (per-call host dispatch cost scales with buffer count).
- Standardization is never materialized: x_std = inv*(x-m) is folded into
  whatever consumes it. Round-1 attention uses row-scaled weight copies
  (Wq' = diag(inv) Wq) on the RAW bf16 inputs, with bias corrections from
  tiny on-device matvecs (bq' = bq - Wq'^T m, bm' = bm_eff - Wm Wv'^T m).
  The K bias is dropped entirely: it shifts each query's scores by a
  constant, which softmax cancels. mlp_dis consumes raw dis input through
  a row-scaled d1; the residual constant shift is absorbed by the
  following BatchNorm (d1_b and m1_b are BN-absorbed no-ops, dropped).
  m1 consumes the raw own-shard p data through row-scaled weights.
- mean_n(xs) == 0 exactly (columns standardized over the mean axis), and
  constant shifts cancel between graphs, so the final MLP collapses to:
  m1 -> BN-stats AllReduce -> relu -> channel-sum AllReduce -> m2 matvec
  on the summed vector -> ||.||^2. The full m2 conv, residual adds, and
  one AllReduce disappear. r2 merge biases are absorbed by m1's BN.
- mlp_dis is replicated over full N on every core (BN stats become
  local), deleting the baseline's stats AllReduce + ds/dt AllGather.
- Round-1 deltas are gated locally (delta * x_std own shard) and
  AllGathered right after each round, hiding the collectives under the
  next attention call's compute.

Attention uses transposed scores (keys on partitions, queries on free) so no
transposes are needed anywhere: scoresT = K_h^T Q_h via one K=64 matmul per
key m-tile; exp on ScalarE (scale=1/8, no max subtraction - scores are O(10));
softmax denominator comes from a ones column prepended to V^T inside the PV
matmul (out partition 0 = denom, 1..64 = data); per-head normalization happens
at PSUM evacuation with a PE-broadcast reciprocal; the merge accumulates per
head with K=64 matmuls so everything stays partition-base-aligned. Head
channels are permuted host-side (d*4+h -> h*64+d) so head slices are
contiguous; merge weight columns permuted to match; V bias folded into the
merge bias (bm_eff = am_b + am_w @ av_b).
"""

import numpy as np
import ml_dtypes

D, H, HD, S, N, EPS = 256, 4, 64, 128, 4096, 1e-5
NC = 8
NQ = N // NC            # 512 queries per core
MT = N // 128           # 32 key m-tiles
HB = HD + 1             # per-head V^T block: [ones | V] = 65 cols

# ---- per-core input shard layout (bf16 [128, SHW]) ----
# The four big tensors are identical on every core, so each core ships only
# its 1/8 column-shard (which doubles as its own q-shard) plus a 1/8 chunk
# of the padded weight block; one AllGather at program start reconstructs
# the full tensors on-device. This matters because per-execution dispatch
# cost scales with input bytes per core (~0.45 ms/MB through the tunnel).
SOPS, SOPT, SODS, SODT, SW = 0, 1024, 2048, 2560, 3072
WCH = 768                # weight chunk cols per core
SHW = SW + WCH           # 3840 bf16 cols per core
WPAD = NC * WCH          # 6144-col padded weight block
# logical col offsets inside the weight block
WOFF = {"wq": 0, "wk": 512, "wv": 1024, "wm": 1536, "m1": 2048,
        "m2": 4096, "d1": 5120, "d2": 5376}

# fp32 bias tensor layout
_FLAY = {"bq": 0, "bm": 2, "m1g": 4, "m1be": 8, "d1g": 12, "d1be": 14,
         "d2b": 16}
WF = 18

_CACHE = {}


def _build_program(dbg=False):
    import contextlib
    import concourse.bass as bass
    import concourse.bacc as bacc
    import concourse.tile as tile
    import concourse.mybir as mybir

    FP32 = mybir.dt.float32
    BF16 = mybir.dt.bfloat16
    AF = mybir.ActivationFunctionType
    ALU = mybir.AluOpType
    AX = mybir.AxisListType

    nc = bacc.Bacc(
        "TRN2",
        target_bir_lowering=False,
        debug=False,
        enable_asserts=False,
        num_devices=NC,
    )

    xin = nc.dram_tensor("xin", [128, SHW], BF16, kind="ExternalInput").ap()
    fin = nc.dram_tensor("fin", [128, WF], FP32, kind="ExternalInput").ap()
    # serialization handle for chained-execution timing: consumed by a DMA,
    # never used in the computation
    chain = nc.dram_tensor("chain", [1, 1], FP32, kind="ExternalInput").ap()
    out_dram = nc.dram_tensor("out", [1, 1], FP32, kind="ExternalOutput").ap()

    RG = [list(range(NC))]
    _dbg_done = set()

    def mkdbg(dma_fn, name, src_ap, shape, dt):
        if not dbg or name in _dbg_done:
            return
        _dbg_done.add(name)
        d = nc.dram_tensor(name, list(shape), dt, kind="ExternalOutput").ap()
        dma_fn(d[tuple(slice(None) for _ in shape)], src_ap)

    with tile.TileContext(nc) as tc:
        st = contextlib.ExitStack()
        PA = st.enter_context(tc.tile_pool(name="persistA", bufs=1))
        PB = st.enter_context(tc.tile_pool(name="persistB", bufs=1))
        Ppr = st.enter_context(tc.tile_pool(name="probs", bufs=4))
        Prd = st.enter_context(tc.tile_pool(name="rdpool", bufs=2))
        Psc = st.enter_context(
            tc.tile_pool(name="psum_sc", bufs=3, space=bass.MemorySpace.PSUM))
        Pout = st.enter_context(
            tc.tile_pool(name="psum_out", bufs=1, space=bass.MemorySpace.PSUM))
        Pbc = st.enter_context(
            tc.tile_pool(name="psum_bc", bufs=1, space=bass.MemorySpace.PSUM))
        Dram = st.enter_context(tc.tile_pool(name="dram", bufs=1, space="DRAM"))

        def pa(name, shape, dt, tag=None):
            return PA.tile(shape, dt, name=name, tag=tag or name)

        def pb(name, shape, dt, tag=None):
            return PB.tile(shape, dt, name=name, tag=tag or name)

        # --- persistent sbuf tensors (raw bf16 inputs stay resident) ---
        xs_bf = pa("xs_bf", [128, 2, 8, 512], BF16)    # raw psT
        xt_bf = pa("xt_bf", [128, 2, 8, 512], BF16)    # raw ptT
        os_raw = pa("os_raw", [128, 2, 512], BF16)     # raw own p shards
        ot_raw = pa("ot_raw", [128, 2, 512], BF16)
        dsr = pb("dsr", [128, 8, 512], BF16)           # raw dsT
        dtr = pb("dtr", [128, 8, 512], BF16)
        ods_r = pa("ods_r", [128, 512], BF16)
        odt_r = pa("odt_r", [128, 512], BF16)
        Qb = pa("Qb", [128, 2, 512], BF16)
        An = pa("An", [64, 4, 512], BF16)       # attn out per head (raw->normed)
        d_bf = pa("d_bf", [128, 2, 512], BF16)  # round-1 delta own
        g_bf = pa("g_bf", [128, 2, 512], BF16)  # gated delta own
        xsd = pa("xsd", [128, 2, 512], BF16)    # std own scratch for gating
        dp0_bf = pa("dp0_bf", [128, 2, 512], BF16)
        dp1_bf = pa("dp1_bf", [128, 2, 512], BF16)
        ds_own_bf = pa("ds_own_bf", [128, 2, 512], BF16)
        dt_own_bf = pa("dt_own_bf", [128, 2, 512], BF16)
        ones = pa("ones", [128, 64], FP32)
        wq_s = pa("wq_s", [128, 2, 256], BF16)
        wk_s = pa("wk_s", [128, 2, 256], BF16)
        wv_s = pa("wv_s", [128, 2, 256], BF16)
        wm_r = pa("wm_r", [64, 4, 256], BF16)   # head-major merge weights
        m1_s = pa("m1_s", [128, 4, 512], BF16)
        m2_s = pa("m2_s", [128, 4, 256], BF16)
        d1_s = pa("d1_s", [128, 256], BF16)
        d2_s = pa("d2_s", [128, 2, 256], BF16)
        # row-scaled weight copies (standardize folded in), per graph s/t
        wq_c = {g: pa(f"wq_c{g}", [128, 2, 256], BF16) for g in "st"}
        wk_c = {g: pa(f"wk_c{g}", [128, 2, 256], BF16) for g in "st"}
        wv_c = {g: pa(f"wv_c{g}", [128, 2, 256], BF16) for g in "st"}
        m1_c = {g: pa(f"m1_c{g}", [128, 2, 512], BF16) for g in "st"}
        d1_c = {g: pa(f"d1_c{g}", [128, 256], BF16) for g in "st"}
        bq_c = {g: pa(f"bq_c{g}", [128, 2, 1], FP32) for g in "st"}
        bm_c = {g: pa(f"bm_c{g}", [128, 2, 1], FP32) for g in "st"}
        m_bf = {g: pa(f"m_bf{g}", [128, 2, 1], BF16) for g in "st"}
        s1b = {g: pa(f"s1b_{g}", [128, 2, 1], BF16) for g in "st"}
        s1h = {g: pa(f"s1h_{g}", [64, 4, 1], BF16) for g in "st"}
        bq_s = pa("bq_s", [128, 2, 1], FP32)
        bm_s = pa("bm_s", [128, 2, 1], FP32)
        m1g_s = pa("m1g_s", [128, 4, 1], FP32)
        m1be_s = pa("m1be_s", [128, 4, 1], FP32)
        d1g_s = pa("d1g_s", [128, 2, 1], FP32)
        d1be_s = pa("d1be_s", [128, 2, 1], FP32)
        d2b_s = pa("d2b_s", [128, 2, 1], FP32)
        bns = pa("bns", [128, 8, 6], FP32)
        parf = pa("parf", [128, 4, 4], FP32)    # m1 stats partials (s:0-1 t:2-3)
        rsum = pa("rsum", [128, 4, 2], FP32)    # relu col sums (s, t)
        totf = pa("totf", [128, 4, 4], FP32)
        tsum = pa("tsum", [128, 4, 2], FP32)
        sdif = pa("sdif", [128, 4, 1], BF16)
        dlt = pa("dlt", [128, 2, 1], FP32)
        dsq = pa("dsq", [128, 2, 1], FP32)
        res = pa("res", [1, 1], FP32)
        chn = pa("chn", [1, 1], FP32)
        sv = pa("sv", [128, 200], FP32)   # scalar scratch, allocator below

        _svc = [0]

        def scol(n=1):
            b = _svc[0]
            _svc[0] += n
            assert _svc[0] <= 200
            return [sv[:, b + i:b + i + 1] for i in range(n)]

        dma = nc.sync.dma_start

        def coll(kind, op, in_t, out_t):
            nc.gpsimd.collective_compute(kind, op, replica_groups=RG,
                                         ins=[in_t.opt()],
                                         outs=[out_t.opt()])

        # ---------------- input AllGather + loads ----------------
        # own shards come straight from this core's input slice
        dma(os_raw[:, :, :],
            xin[:, SOPS:SOPS + 1024].rearrange("p (g f) -> p g f", g=2))
        dma(ot_raw[:, :, :],
            xin[:, SOPT:SOPT + 1024].rearrange("p (g f) -> p g f", g=2))
        dma(ods_r[:, :], xin[:, SODS:SODS + 512])
        dma(odt_r[:, :], xin[:, SODT:SODT + 512])
        # full tensors + weights reconstructed from the gathered shards
        agi = Dram.tile([128, SHW], BF16, name="agi")
        ago = Dram.tile([NC * 128, SHW], BF16, name="ago",
                        addr_space="Shared")
        dma(agi[:, :], xin[:, :])
        coll("AllGather", ALU.bypass, agi, ago)
        for r in range(NC):
            rs = slice(r * 128, (r + 1) * 128)
            for g in range(2):
                dma(xs_bf[:, g, r, :], ago[rs, SOPS + g * 512:
                                           SOPS + (g + 1) * 512])
                dma(xt_bf[:, g, r, :], ago[rs, SOPT + g * 512:
                                           SOPT + (g + 1) * 512])
            dma(dsr[:, r, :], ago[rs, SODS:SODS + 512])
            dma(dtr[:, r, :], ago[rs, SODT:SODT + 512])

        def wld(dst, nm, gcols, ngroups, flat=False):
            # dst[:, g, a:b] <- weight-block cols [base+g*gcols+a, ...+b)
            base = WOFF[nm]
            for g in range(ngroups):
                lo, hi = base + g * gcols, base + (g + 1) * gcols
                a = lo
                while a < hi:
                    ch = a // WCH
                    b = min(hi, (ch + 1) * WCH)
                    src = ago[ch * 128:(ch + 1) * 128,
                              SW + a - ch * WCH:SW + b - ch * WCH]
                    if flat:
                        dma(dst[:, a - lo:b - lo], src)
                    else:
                        dma(dst[:, g, a - lo:b - lo], src)
                    a = b

        wld(wq_s, "wq", 256, 2)
        wld(wk_s, "wk", 256, 2)
        wld(wv_s, "wv", 256, 2)
        wld(m1_s, "m1", 512, 4)
        wld(m2_s, "m2", 256, 4)
        wld(d1_s, "d1", 256, 1, flat=True)
        wld(d2_s, "d2", 256, 2)
        for h in range(H):
            ch = (WOFF["wm"] + (h // 2) * 256) // WCH
            co = WOFF["wm"] + (h // 2) * 256 - ch * WCH
            dma(wm_r[:, h, :],
                ago[ch * 128 + (h % 2) * 64:ch * 128 + (h % 2) * 64 + 64,
                    SW + co:SW + co + 256])

        def fld(dst, nm, g):
            o = _FLAY[nm]
            dma(dst[:, :, :], fin[:, o:o + g].rearrange("p (g c) -> p g c", c=1))

        fld(bq_s, "bq", 2)
        fld(bm_s, "bm", 2)
        fld(m1g_s, "m1g", 4)
        fld(m1be_s, "m1be", 4)
        fld(d1g_s, "d1g", 2)
        fld(d1be_s, "d1be", 2)
        fld(d2b_s, "d2b", 2)
        dma(chn[:, :], chain[:, :])
        nc.gpsimd.memset(ones[:, :], 1.0)

        def inv_std(var_ap, eps):
            t, s0, r0, s1, inv = scol(5)
            nc.vector.tensor_scalar_add(t, var_ap, float(eps))
            nc.scalar.activation(s0, t, AF.Sqrt)
            nc.vector.reciprocal(r0, s0)
            nc.vector.tensor_mul(r0, t, r0)
            nc.vector.tensor_add(r0, r0, s0)
            nc.vector.tensor_scalar_mul(s1, r0, 0.5)
            nc.vector.reciprocal(inv, s1)
            return inv

        # ---- p stats + folded weight prep (per graph g in {s,t}) ----
        invp, nbp = {}, {}
        for g, praw in (("s", xs_bf), ("t", xt_bf)):
            for cg in range(2):
                for c in range(8):
                    nc.vector.bn_stats(bns[:, c, :], praw[:, cg, c, :])
                ag2 = pa(f"ag_{g}{cg}", [128, 2], FP32)
                nc.vector.bn_aggr(ag2[:, :], bns[:, :, :])
                inv = inv_std(ag2[:, 1:2], 0.0)
                (nb,) = scol(1)
                nc.vector.tensor_mul(nb, ag2[:, 0:1], inv)
                nc.vector.tensor_scalar_mul(nb, nb, -1.0)
                invp[(g, cg)], nbp[(g, cg)] = inv, nb
                nc.vector.tensor_copy(m_bf[g][:, cg, :], ag2[:, 0:1])
                nc.vector.tensor_scalar_mul(wq_c[g][:, cg, :], wq_s[:, cg, :],
                                            inv)
                nc.vector.tensor_scalar_mul(wk_c[g][:, cg, :], wk_s[:, cg, :],
                                            inv)
                nc.vector.tensor_scalar_mul(wv_c[g][:, cg, :], wv_s[:, cg, :],
                                            inv)
                nc.vector.tensor_scalar_mul(m1_c[g][:, cg, :], m1_s[:, cg, :],
                                            inv)
            # bias corrections: bq' = bq - Wq'^T m ; bm' = bm - Wm (Wv'^T m)
            qsh = Psc.tile([128, 2, 512], FP32, tag="sc", name=f"qsh_{g}")
            for og in range(2):
                for cg in range(2):
                    nc.tensor.matmul(qsh[:, og, 0:1],
                                     wq_c[g][:, cg, og * 128:(og + 1) * 128],
                                     m_bf[g][:, cg, :],
                                     start=(cg == 0), stop=(cg == 1))
            for og in range(2):
                nc.vector.tensor_sub(bq_c[g][:, og, :], bq_s[:, og, :],
                                     qsh[:, og, 0:1])
            vsh = Psc.tile([128, 2, 512], FP32, tag="sc", name=f"vsh_{g}")
            for og in range(2):
                for cg in range(2):
                    nc.tensor.matmul(vsh[:, og, 0:1],
                                     wv_c[g][:, cg, og * 128:(og + 1) * 128],
                                     m_bf[g][:, cg, :],
                                     start=(cg == 0), stop=(cg == 1))
            for og in range(2):
                nc.vector.tensor_copy(s1b[g][:, og, :], vsh[:, og, 0:1])
            for h in range(H):
                dma(s1h[g][:, h, :],
                    s1b[g][(h % 2) * 64:(h % 2) * 64 + 64, h // 2, :])
            msh = Psc.tile([128, 2, 512], FP32, tag="sc", name=f"msh_{g}")
            for og in range(2):
                for h in range(H):
                    nc.tensor.matmul(
                        msh[:, og, 0:1], wm_r[:, h, og * 128:(og + 1) * 128],
                        s1h[g][:, h, :],
                        start=(h == 0), stop=(h == 3))
            for og in range(2):
                nc.vector.tensor_sub(bm_c[g][:, og, :], bm_s[:, og, :],
                                     msh[:, og, 0:1])

        # ---------------- attention ----------------
        VT = pb("VT", [128, MT, H * HB], BF16)
        Kb = pb("Kb", [128, 2, 8, 512], BF16)
        ds_full = pb("ds_full", [128, 2, 8, 512], BF16)
        dt_full = pb("dt_full", [128, 2, 8, 512], BF16)
        d0f = pa("d0f", [128, 2, 8, 512], BF16, tag="xs_bf")
        for h in range(H):
            nc.gpsimd.memset(VT[:, :, h * HB + HD], 1.0)

        def attention(tag, q_own, k_src, v_src, out_bf, wq, bq, wk, wv, bm):
            # Q projection (+bias)
            qp = Psc.tile([128, 2, 512], FP32, tag="sc", name=f"qp_{tag}")
            for og in range(2):
                for cg in range(2):
                    nc.tensor.matmul(qp[:, og, :],
                                     wq[:, cg, og * 128:(og + 1) * 128],
                                     q_own[:, cg, :],
                                     start=(cg == 0), stop=(cg == 1))
            for og in range(2):
                nc.vector.tensor_scalar_add(Qb[:, og, :], qp[:, og, :],
                                            bq[:, og, :])
            # K projection, full N (no bias: softmax-invariant)
            for og in range(2):
                for c in range(8):
                    kp = Psc.tile([128, 2, 512], FP32, tag="sc", name=f"kp_{tag}")
                    for cg in range(2):
                        nc.tensor.matmul(kp[:, 0, :],
                                         wk[:, cg, og * 128:(og + 1) * 128],
                                         k_src[:, cg, c, :],
                                         start=(cg == 0), stop=(cg == 1))
                    nc.vector.tensor_copy(Kb[:, og, c, :], kp[:, 0, :])
            # V^T projection (keys on partitions), bias folded into bm
            for m in range(MT):
                c, f0 = divmod(m * 128, 512)
                vp = Psc.tile([128, 2, 512], FP32, tag="sc", name=f"vp_{tag}")
                for cg in range(2):
                    nc.tensor.matmul(vp[:, 0, 0:256],
                                     v_src[:, cg, c, f0:f0 + 128],
                                     wv[:, cg, :],
                                     start=(cg == 0), stop=(cg == 1))
                nc.vector.tensor_copy(
                    VT[:, m, :].rearrange("p (h c) -> p h c", c=HB)[:, :, 0:HD],
                    vp[:, 0, 0:256].rearrange("p (h c) -> p h c", c=HD))
            # streaming attention per head, PV pipelined one group behind
            recs = []
            for h in range(H):
                hg, hp = h // 2, (h % 2) * 64
                op = Pout.tile([65, 512], FP32, tag="out", name=f"op_{tag}{h}")
                prev = None
                for g in range(16):
                    sc = Psc.tile([128, 2, 512], FP32, tag="sc", name=f"sc_{tag}")
                    for j in range(2):
                        m = g * 2 + j
                        c, f0 = divmod(m * 128, 512)
                        nc.tensor.matmul(sc[:, j, :],
                                         Kb[hp:hp + 64, hg, c, f0:f0 + 128],
                                         Qb[hp:hp + 64, hg, :],
                                         start=True, stop=True)
                    pr = Ppr.tile([128, 2, 512], BF16, tag="pr", name=f"pr_{tag}",
                                  bufs=2)
                    nc.scalar.activation(pr[:, :, :], sc[:, :, :], AF.Exp,
                                         scale=0.125)
                    if prev is not None:
                        for j in range(2):
                            m = prev[0] * 2 + j
                            nc.tensor.matmul(
                                op[:, :], VT[:, m, h * HB:(h + 1) * HB],
                                prev[1][:, j, :], start=(m == 0), stop=False)
                    prev = (g, pr)
                for j in range(2):
                    m = prev[0] * 2 + j
                    nc.tensor.matmul(op[:, :], VT[:, m, h * HB:(h + 1) * HB],
                                     prev[1][:, j, :], start=False,
                                     stop=(m == MT - 1))
                # evacuate raw numerator (bf16); denominator: psum row 64 ->
                # sbuf row 64 (ScalarE) -> partition 0 (DMA) -> reciprocal at
                # base 0 (custom DVE op misbehaves at base 64). The PE
                # broadcast + normalize are deferred past the head loop so the
                # PE never stalls on this chain.
                nc.vector.tensor_copy(An[:, h, :], op[0:64, :])
                dnm = Ppr.tile([65, 512], FP32, tag="dnm", bufs=2,
                               name=f"dnm_{tag}{h}")
                nc.scalar.activation(dnm[64:65, :], op[64:65, :], AF.Copy)
                dma(dnm[0:1, :], dnm[64:65, :])
                rc = Ppr.tile([1, 512], FP32, tag="rc", bufs=4,
                              name=f"rc_{tag}{h}")
                nc.vector.reciprocal_approx_fast(rc[0:1, :], dnm[0:1, :])
                recs.append(rc)
            for h in range(H):
                bc = Pbc.tile([64, 512], FP32, tag="bc", name=f"bc_{tag}{h}")
                nc.tensor.matmul(bc[:, :], ones[0:1, 0:64], recs[h][0:1, :],
                                 start=True, stop=True)
                nc.vector.tensor_mul(An[:, h, :], An[:, h, :], bc[:, :])
            # merge: accumulate per head (K=64); bias only when not absorbed
            mg = Psc.tile([128, 2, 512], FP32, tag="sc", name=f"mg_{tag}")
            for og in range(2):
                for h in range(H):
                    nc.tensor.matmul(mg[:, og, :],
                                     wm_r[:, h, og * 128:(og + 1) * 128],
                                     An[:, h, :],
                                     start=(h == 0), stop=(h == 3))
            for og in range(2):
                if bm is not None:
                    nc.vector.tensor_scalar_add(out_bf[:, og, :], mg[:, og, :],
                                                bm[:, og, :])
                else:
                    nc.vector.tensor_copy(out_bf[:, og, :], mg[:, og, :])

        def gate_and_send(delta, oraw, gkey, agin):
            # g = delta * std(own raw);  std = inv*(raw) + nb  per cg group
            for cg in range(2):
                nc.scalar.activation(xsd[:, cg, :], oraw[:, cg, :],
                                     AF.Identity, bias=nbp[(gkey, cg)],
                                     scale=invp[(gkey, cg)])
                nc.vector.tensor_mul(g_bf[:, cg, :], delta[:, cg, :],
                                     xsd[:, cg, :])
                dma(agin[cg * 128:(cg + 1) * 128, :], g_bf[:, cg, :])

        # ---------------- round 1a ----------------
        ag0_in = Dram.tile([D, NQ], BF16, name="ag0_in")
        ag0_out = Dram.tile([NC * D, NQ], BF16, name="ag0_out",
                            addr_space="Shared")
        ag1_in = Dram.tile([D, NQ], BF16, name="ag1_in")
        ag1_out = Dram.tile([NC * D, NQ], BF16, name="ag1_out",
                            addr_space="Shared")
        attention("r1a", os_raw, xt_bf, xt_bf, d_bf,
                  wq_c["s"], bq_c["s"], wk_c["t"], wv_c["t"], bm_c["t"])
        mkdbg(dma, "dbg_dbf", d_bf[:, :, :], (128, 2, 512), BF16)
        gate_and_send(d_bf, ot_raw, "t", ag0_in)
        coll("AllGather", ALU.bypass, ag0_in, ag0_out)

        # ---- dis stats + scaled d1 ----
        for g, draw in (("s", dsr), ("t", dtr)):
            for c in range(8):
                nc.vector.bn_stats(bns[:, c, :], draw[:, c, :])
            ag2 = pa(f"agd_{g}", [128, 2], FP32)
            nc.vector.bn_aggr(ag2[:, :], bns[:, :, :])
            inv = inv_std(ag2[:, 1:2], 0.0)
            nc.vector.tensor_scalar_mul(d1_c[g][:, :], d1_s[:, :], inv)

        # ---- mlp_dis replicated (full N, local BN stats) + own shard ----
        x1o = pb("x1o", [128, 2, 512], BF16)
        rdo = pb("rdo", [128, 2, 512], BF16)
        for g, draw, odr, dfull, downb in (
                ("s", dsr, ods_r, ds_full, ds_own_bf),
                ("t", dtr, odt_r, dt_full, dt_own_bf)):
            x1fl = pb(f"x1f_{g}", [128, 2, 8, 512], BF16, tag="x1f")
            for og in range(2):
                for c in range(8):
                    mp = Psc.tile([128, 2, 512], FP32, tag="sc", name="mp_d1")
                    nc.tensor.matmul(mp[:, 0, :],
                                     d1_c[g][:, og * 128:(og + 1) * 128],
                                     draw[:, c, :], start=True, stop=True)
                    nc.vector.tensor_copy(x1fl[:, og, c, :], mp[:, 0, :])
                mpo = Psc.tile([128, 2, 512], FP32, tag="sc", name="mp_d1o")
                nc.tensor.matmul(mpo[:, 0, :],
                                 d1_c[g][:, og * 128:(og + 1) * 128],
                                 odr[:, :], start=True, stop=True)
                nc.vector.tensor_copy(x1o[:, og, :], mpo[:, 0, :])
            ab = []
            for og in range(2):
                for c in range(8):
                    nc.vector.bn_stats(bns[:, c, :], x1fl[:, og, c, :])
                ag2 = pa(f"agx_{g}{og}", [128, 2], FP32)
                nc.vector.bn_aggr(ag2[:, :], bns[:, :, :])
                inv = inv_std(ag2[:, 1:2], EPS)
                a_, b_ = scol(2)
                nc.vector.tensor_mul(a_, d1g_s[:, og, :], inv)
                nc.vector.tensor_mul(b_, ag2[:, 0:1], a_)
                nc.vector.tensor_scalar_mul(b_, b_, -1.0)
                nc.vector.tensor_add(b_, b_, d1be_s[:, og, :])
                ab.append((a_, b_))
                nc.scalar.activation(rdo[:, og, :], x1o[:, og, :],
                                     AF.Relu, bias=b_, scale=a_)
            for c in range(8):
                rdc = Prd.tile([128, 2, 512], BF16, tag="rdc",
                               name=f"rdc_{g}{c}")
                for cg in range(2):
                    nc.scalar.activation(rdc[:, cg, :],
                                         x1fl[:, cg, c, :],
                                         AF.Relu, bias=ab[cg][1],
                                         scale=ab[cg][0])
                for og in range(2):
                    mp = Psc.tile([128, 2, 512], FP32, tag="sc",
                                  name="mp_d2")
                    for cg in range(2):
                        nc.tensor.matmul(
                            mp[:, 0, :],
                            d2_s[:, cg, og * 128:(og + 1) * 128],
                            rdc[:, cg, :],
                            start=(cg == 0), stop=(cg == 1))
                    nc.vector.tensor_scalar_add(dfull[:, og, c, :],
                                                mp[:, 0, :],
                                                d2b_s[:, og, :])
            for og in range(2):
                mpo = Psc.tile([128, 2, 512], FP32, tag="sc", name="mp_d2o")
                for cg in range(2):
                    nc.tensor.matmul(mpo[:, 0, :],
                                     d2_s[:, cg, og * 128:(og + 1) * 128],
                                     rdo[:, cg, :], start=(cg == 0),
                                     stop=(cg == 1))
                nc.vector.tensor_scalar_add(downb[:, og, :], mpo[:, 0, :],
                                            d2b_s[:, og, :])
        mkdbg(dma, "dbg_dsown", ds_own_bf[:, :, :], (128, 2, 512), BF16)
        mkdbg(dma, "dbg_dsfull", ds_full[:, :, :, :], (128, 2, 8, 512), BF16)

        # ---------------- round 1b ----------------
        attention("r1b", ot_raw, xs_bf, xs_bf, d_bf,
                  wq_c["t"], bq_c["t"], wk_c["s"], wv_c["s"], bm_c["s"])
        gate_and_send(d_bf, os_raw, "s", ag1_in)
        coll("AllGather", ALU.bypass, ag1_in, ag1_out)

        # gather gated delta0 (values for r2a)
        for r in range(NC):
            for g in range(2):
                dma(d0f[:, g, r, :],
                    ag0_out[r * D + g * 128:r * D + (g + 1) * 128, :])
        mkdbg(dma, "dbg_d0f", d0f[:, :, :, :], (128, 2, 8, 512), BF16)

        # ---------------- round 2a + m1_s ----------------
        attention("r2a", ds_own_bf, dt_full, d0f, dp0_bf,
                  wq_s, bq_s, wk_s, wv_s, None)
        mkdbg(dma, "dbg_dp0", dp0_bf[:, :, :], (128, 2, 512), BF16)

        x1_s = pb("x1_s", [128, 4, 512], BF16, tag="dsr")
        x1_t = pb("x1_t", [128, 4, 512], BF16, tag="Kb")
        sq = pb("sq", [128, 4, 512], BF16, tag="x1f")

        def m1_block(x1, oraw, dp, gkey, pcol):
            rhs = [oraw[:, 0, :], oraw[:, 1, :], dp[:, 0, :], dp[:, 1, :]]
            for og in range(4):
                mp = Psc.tile([128, 2, 512], FP32, tag="sc", name=f"mp_m1{gkey}")
                for cg in range(4):
                    lhs = (m1_c[gkey][:, cg, og * 128:(og + 1) * 128]
                           if cg < 2 else
                           m1_s[:, cg, og * 128:(og + 1) * 128])
                    nc.tensor.matmul(mp[:, 0, :], lhs, rhs[cg],
                                     start=(cg == 0), stop=(cg == 3))
                nc.vector.tensor_copy(x1[:, og, :], mp[:, 0, :])
            nc.vector.reduce_sum(parf[:, :, pcol], x1[:, :, :], axis=AX.X)
            nc.vector.tensor_mul(sq[:, :, :], x1[:, :, :], x1[:, :, :])
            nc.vector.reduce_sum(parf[:, :, pcol + 1], sq[:, :, :], axis=AX.X)

        m1_block(x1_s, os_raw, dp0_bf, "s", 0)
        ars_in = Dram.tile([2 * D, 2], FP32, name="ars_in")
        ars_out = Dram.tile([2 * D, 2], FP32, name="ars_out",
                            addr_space="Shared")
        for g in range(4):
            dma(ars_in[g * 128:(g + 1) * 128, :], parf[:, g, 0:2])
        coll("AllReduce", ALU.add, ars_in, ars_out)

        # gather gated delta1 (values for r2b) into the xt_bf slot
        pt_f = pa("pt_f", [128, 2, 8, 512], BF16, tag="xt_bf")
        for r in range(NC):
            for g in range(2):
                dma(pt_f[:, g, r, :],
                    ag1_out[r * D + g * 128:r * D + (g + 1) * 128, :])

        # ---------------- round 2b + m1_t + final ----------------
        attention("r2b", dt_own_bf, ds_full, pt_f, dp1_bf,
                  wq_s, bq_s, wk_s, wv_s, None)
        mkdbg(dma, "dbg_dp1", dp1_bf[:, :, :], (128, 2, 512), BF16)

        r_f = pb("r_f", [128, 4, 512], BF16, tag="dtr")
        for g in range(4):
            dma(totf[:, g, 0:2], ars_out[g * 128:(g + 1) * 128, :])

        def bn_apply_relu_sum(x1, c0, sum_col):
            for og in range(4):
                mu, va, msq, a_, b_ = scol(5)
                nc.vector.tensor_scalar_mul(mu, totf[:, og, c0:c0 + 1],
                                            1.0 / N)
                nc.vector.tensor_scalar_mul(va, totf[:, og, c0 + 1:c0 + 2],
                                            1.0 / N)
                nc.vector.tensor_mul(msq, mu, mu)
                nc.vector.tensor_sub(va, va, msq)
                inv = inv_std(va, EPS)
                nc.vector.tensor_mul(a_, m1g_s[:, og, :], inv)
                nc.vector.tensor_mul(b_, mu, a_)
                nc.vector.tensor_scalar_mul(b_, b_, -1.0)
                nc.vector.tensor_add(b_, b_, m1be_s[:, og, :])
                nc.scalar.activation(r_f[:, og, :], x1[:, og, :], AF.Relu,
                                     bias=b_, scale=a_)
            nc.vector.reduce_sum(rsum[:, :, sum_col], r_f[:, :, :], axis=AX.X)

        bn_apply_relu_sum(x1_s, 0, 0)
        m1_block(x1_t, ot_raw, dp1_bf, "t", 2)

        # AllReduce #2: [stats_t (2 cols) | sums_s (1 col)]
        ar2_in = Dram.tile([2 * D, 3], FP32, name="ar2_in")
        ar2_out = Dram.tile([2 * D, 3], FP32, name="ar2_out",
                            addr_space="Shared")
        for g in range(4):
            dma(ar2_in[g * 128:(g + 1) * 128, 0:2], parf[:, g, 2:4])
            dma(ar2_in[g * 128:(g + 1) * 128, 2:3], rsum[:, g, 0:1])
        coll("AllReduce", ALU.add, ar2_in, ar2_out)
        for g in range(4):
            dma(totf[:, g, 2:4], ar2_out[g * 128:(g + 1) * 128, 0:2])
            dma(tsum[:, g, 0:1], ar2_out[g * 128:(g + 1) * 128, 2:3])

        bn_apply_relu_sum(x1_t, 2, 1)

        # AllReduce #3: sums_t
        ar3_in = Dram.tile([2 * D, 1], FP32, name="ar3_in")
        ar3_out = Dram.tile([2 * D, 1], FP32, name="ar3_out",
                            addr_space="Shared")
        for g in range(4):
            dma(ar3_in[g * 128:(g + 1) * 128, :], rsum[:, g, 1:2])
        coll("AllReduce", ALU.add, ar3_in, ar3_out)
        for g in range(4):
            dma(tsum[:, g, 1:2], ar3_out[g * 128:(g + 1) * 128, :])

        # delta = m2 @ (S_s - S_t)/N ; result = ||delta||^2
        for g in range(4):
            df, = scol(1)
            nc.vector.tensor_sub(df, tsum[:, g, 0:1], tsum[:, g, 1:2])
            nc.vector.tensor_scalar_mul(df, df, 1.0 / N)
            nc.vector.tensor_copy(sdif[:, g, :], df)
        mpv = Psc.tile([128, 2, 512], FP32, tag="sc", name="mpv")
        for og in range(2):
            for cg in range(4):
                nc.tensor.matmul(mpv[:, og, 0:1],
                                 m2_s[:, cg, og * 128:(og + 1) * 128],
                                 sdif[:, cg, :],
                                 start=(cg == 0), stop=(cg == 3))
        for og in range(2):
            nc.vector.tensor_copy(dlt[:, og, :], mpv[:, og, 0:1])
        nc.vector.tensor_mul(dsq[:, :, :], dlt[:, :, :], dlt[:, :, :])
        dot = Pbc.tile([64, 512], FP32, tag="bc", name="dot")
        for g in range(2):
            nc.tensor.matmul(dot[0:1, 0:1], dsq[:, g, :], ones[:, 0:1],
                             start=(g == 0), stop=(g == 1))
        nc.vector.tensor_copy(res[:, :], dot[0:1, 0:1])
        dma(out_dram[:, :], res[:, :])

        st.close()

    nc.compile()
    return nc


# head permutation: new row i = h*64+d  <- old channel d*4+h
_PERM = np.array([d * H + h for h in range(H) for d in range(HD)])


def _prep_inputs(inputs):
    bf16 = ml_dtypes.bfloat16
    f32 = np.float32

    def C(x, dt=f32):
        return np.ascontiguousarray(np.asarray(x), dtype=dt)

    p_src = C(inputs["p_src"])[0]
    p_tar = C(inputs["p_tar"])[0]
    dis_src = C(inputs["dis_src"])[0]
    dis_tar = C(inputs["dis_tar"])[0]
    aq_w = C(inputs["aq_w"]); ak_w = C(inputs["ak_w"])
    av_w = C(inputs["av_w"]); am_w = C(inputs["am_w"])

    wpad = np.zeros((128, WPAD), bf16)

    def putw(nm, arr):
        a = np.asarray(arr, bf16)
        R, Cc = a.shape
        o = WOFF[nm]
        for g in range(R // 128):
            wpad[:, o + g * Cc:o + (g + 1) * Cc] = a[g * 128:(g + 1) * 128, :]

    putw("wq", aq_w[_PERM, :].T)
    putw("wk", ak_w[_PERM, :].T)
    putw("wv", av_w[_PERM, :].T)
    putw("wm", am_w[:, _PERM].T)   # head-major rows -> 2 slabs (h01 | h23)
    putw("m1", C(inputs["m1_w"]).T)
    putw("m2", C(inputs["m2_w"]).T)
    putw("d1", C(inputs["d1_w"]).T)
    putw("d2", C(inputs["d2_w"]).T)

    fb = np.zeros((128, WF), f32)

    def putf(nm, vec):
        v = np.asarray(vec, f32).reshape(-1)
        o = _FLAY[nm]
        for g in range(v.size // 128):
            fb[:, o + g] = v[g * 128:(g + 1) * 128]

    putf("bq", C(inputs["aq_b"])[_PERM])
    putf("bm", C(inputs["am_b"]) + am_w @ C(inputs["av_b"]))
    putf("m1g", inputs["m1_g"])
    putf("m1be", inputs["m1_be"])
    putf("d1g", inputs["d1_g"])
    putf("d1be", inputs["d1_be"])
    putf("d2b", inputs["d2_b"])

    in_maps = []
    for c in range(NC):
        sl = slice(c * NQ, (c + 1) * NQ)
        x = np.zeros((128, SHW), bf16)
        for o, src in ((SOPS, p_src), (SOPT, p_tar)):
            own = np.asarray(src[sl, :].T, bf16)
            x[:, o:o + 512] = own[0:128]
            x[:, o + 512:o + 1024] = own[128:256]
        x[:, SODS:SODS + 512] = np.asarray(dis_src[sl, :].T, bf16)
        x[:, SODT:SODT + 512] = np.asarray(dis_tar[sl, :].T, bf16)
        x[:, SW:SHW] = wpad[:, c * WCH:(c + 1) * WCH]
        in_maps.append({"xin": x, "fin": fb,
                        "chain": np.zeros((1, 1), f32)})
    return in_maps


def kernel(**inputs):
    from concourse.bass_utils import run_bass_kernel_spmd

    if "nc" not in _CACHE:
        _CACHE["nc"] = _build_program()
    nc = _CACHE["nc"]
    in_maps = _prep_inputs(inputs)
    res = run_bass_kernel_spmd(nc, in_maps, core_ids=list(range(NC)))
    return np.asarray(res.results[0]["out"], np.float32).reshape(())


# revision 46
# speedup vs baseline: 27.0612x; 1.0065x over previous
"""Trainium2 Bass kernel for nn_AttentionalGNN (self-contained).

  xs/xt = standardize(p_src/p_tar).T ; ds/dt = mlp_dis(standardize(dis).T)
  delta0 = attn(xs, xt, xt); delta1 = attn(xt, xs, xs)
  ps = delta0*xt; pt = delta1*xs
  delta0' = attn(ds, dt, ps); delta1' = attn(dt, ds, pt)
  out_s = xs + mlp(cat(xs, delta0')); out_t likewise
  return ||mean_n(out_s) - mean_n(out_t)||^2

8-core SPMD, queries sharded 512/core for all four attention calls
(keys/values replicated). Structural choices:

- Each core ships only its 1/8 input shard (~1 MB); one AllGather at
  program start reconstructs the replicated tensors on-device (per-exec
  dispatch cost scales with input bytes/core through the PJRT tunnel).
- Standardization is never materialized: x_std = inv*(x-m) is folded into
  whatever consumes it. Round-1 attention uses row-scaled weight copies
  (Wq' = diag(inv) Wq) on the RAW bf16 inputs, with bias corrections from
  tiny on-device matvecs (bq' = bq - Wq'^T m, bm' = bm_eff - Wm Wv'^T m).
  The K bias is dropped: it shifts each query's scores by a constant,
  which softmax cancels. mlp_dis consumes raw dis input through a
  row-scaled d1; the residual shift is absorbed by the following
  BatchNorm (d1_b and m1_b are BN-absorbed no-ops, dropped). m1 consumes
  the raw own-shard p data through row-scaled first-half weights.
- Standardize statistics come from own-shard partial sums + one tiny
  AllReduce issued before the input AllGather, so inv/mean are ready
  before the gathered data lands.
- mean_n(xs) == 0 exactly (columns standardized over the mean axis), so
  the final MLP collapses to: m1 -> BN-stats AllReduce -> relu ->
  channel-sum AllReduce -> m2 matvec on the summed vector -> ||.||^2.
- mlp_dis is replicated over full N on every core (BN stats local),
  deleting the baseline's stats AllReduce + ds/dt AllGather. Its d1
  halves run before r1a / between r1a+r1b, its BN+d2 in the r1a->r1b
  gap, so the serial BN chain hides under attention compute.
- Round-1 deltas are gated locally (delta * x_std own shard) and
  AllGathered right after each round, hiding under the next attention.
- All rsqrt on DVE (magic seed + Newton) - ScalarE Sqrt would thrash the
  ACT table set against Exp (~5.3us per switch mid-attention).

Attention uses transposed scores (keys on partitions, queries on free) so no
transposes are needed anywhere: scoresT = K_h^T Q_h via one K=64 matmul per
key m-tile; exp on ScalarE (scale=1/8, no max subtraction - scores are O(10));
softmax denominator comes from a ones column appended to V^T inside the PV
matmul (out partition 64 = denom); the reciprocal runs on DVE directly from
PSUM partition 64 and feeds a PE broadcast, deferred past the head loop so
the PE never stalls on the chain. Head channels are permuted host-side
(d*4+h -> h*64+d) so head slices are contiguous; merge weight columns
permuted to match; V bias folded into the merge bias.
"""

import numpy as np
import ml_dtypes

D, H, HD, S, N, EPS = 256, 4, 64, 128, 4096, 1e-5
NC = 8
NQ = N // NC            # 512 queries per core
MT = N // 128           # 32 key m-tiles
HB = HD + 1             # per-head V^T block: [V | ones] = 65 cols

# ---- per-core input shard layout (bf16 [128, SHW]) ----
SOPS, SOPT, SODS, SODT, SW = 0, 1024, 2048, 2560, 3072
WCH = 768                # weight chunk cols per core
SHW = SW + WCH           # 3840 bf16 cols per core
WPAD = NC * WCH          # 6144-col padded weight block
WOFF = {"wq": 0, "wk": 512, "wv": 1024, "wm": 1536, "m1": 2048,
        "m2": 4096, "d1": 5120, "d2": 5376}

# fp32 bias tensor layout
_FLAY = {"bq": 0, "bm": 2, "m1g": 4, "m1be": 8, "d1g": 12, "d1be": 14,
         "d2b": 16}
WF = 18

_CACHE = {}


def _build_program(dbg=False):
    import contextlib
    import concourse.bass as bass
    import concourse.bacc as bacc
    import concourse.tile as tile
    import concourse.mybir as mybir

    FP32 = mybir.dt.float32
    BF16 = mybir.dt.bfloat16
    I32 = mybir.dt.int32
    AF = mybir.ActivationFunctionType
    ALU = mybir.AluOpType
    AX = mybir.AxisListType

    nc = bacc.Bacc(
        "TRN2",
        target_bir_lowering=False,
        debug=False,
        enable_asserts=False,
        num_devices=NC,
    )

    xin = nc.dram_tensor("xin", [128, SHW], BF16, kind="ExternalInput").ap()
    fin = nc.dram_tensor("fin", [128, WF], FP32, kind="ExternalInput").ap()
    # serialization handle for chained-execution timing: consumed by a DMA,
    # never used in the computation
    chain = nc.dram_tensor("chain", [1, 1], FP32, kind="ExternalInput").ap()
    out_dram = nc.dram_tensor("out", [1, 1], FP32, kind="ExternalOutput").ap()

    RG = [list(range(NC))]
    _dbg_done = set()

    def mkdbg(dma_fn, name, src_ap, shape, dt):
        if not dbg or name in _dbg_done:
            return
        _dbg_done.add(name)
        d = nc.dram_tensor(name, list(shape), dt, kind="ExternalOutput").ap()
        dma_fn(d[tuple(slice(None) for _ in shape)], src_ap)

    with tile.TileContext(nc) as tc:
        st = contextlib.ExitStack()
        PA = st.enter_context(tc.tile_pool(name="persistA", bufs=1))
        PB = st.enter_context(tc.tile_pool(name="persistB", bufs=1))
        Ppr = st.enter_context(tc.tile_pool(name="probs", bufs=4))
        Prd = st.enter_context(tc.tile_pool(name="rdpool", bufs=2))
        Psc = st.enter_context(
            tc.tile_pool(name="psum_sc", bufs=3, space=bass.MemorySpace.PSUM))
        Pout = st.enter_context(
            tc.tile_pool(name="psum_out", bufs=1, space=bass.MemorySpace.PSUM))
        Pbc = st.enter_context(
            tc.tile_pool(name="psum_bc", bufs=1, space=bass.MemorySpace.PSUM))
        Dram = st.enter_context(tc.tile_pool(name="dram", bufs=1, space="DRAM"))

        def pa(name, shape, dt, tag=None):
            return PA.tile(shape, dt, name=name, tag=tag or name)

        def pb(name, shape, dt, tag=None):
            return PB.tile(shape, dt, name=name, tag=tag or name)

        # --- persistent sbuf tensors (raw bf16 inputs stay resident) ---
        xs_bf = pa("xs_bf", [128, 2, 8, 512], BF16)    # raw psT
        xt_bf = pa("xt_bf", [128, 2, 8, 512], BF16)    # raw ptT
        os_raw = pa("os_raw", [128, 2, 512], BF16)     # raw own p shards
        ot_raw = pa("ot_raw", [128, 2, 512], BF16)
        dsr = pb("dsr", [128, 8, 512], BF16)           # raw dsT
        dtr = pb("dtr", [128, 8, 512], BF16)
        ods_r = pa("ods_r", [128, 512], BF16)
        odt_r = pa("odt_r", [128, 512], BF16)
        Qb = pa("Qb", [128, 2, 512], BF16)
        An = pa("An", [64, 4, 512], BF16)       # attn out per head (raw->normed)
        d_bf = pa("d_bf", [128, 2, 512], BF16)  # round-1 delta own
        g_bf = pa("g_bf", [128, 2, 512], BF16)  # gated delta own
        xsd = pa("xsd", [128, 2, 512], BF16)    # std own scratch for gating
        dp0_bf = pa("dp0_bf", [128, 2, 512], BF16)
        dp1_bf = pa("dp1_bf", [128, 2, 512], BF16)
        ds_own_bf = pa("ds_own_bf", [128, 2, 512], BF16, tag="g_bf")
        dt_own_bf = pa("dt_own_bf", [128, 2, 512], BF16)
        ones = pa("ones", [128, 64], FP32)
        wq_s = pa("wq_s", [128, 2, 256], BF16)
        wk_s = pa("wk_s", [128, 2, 256], BF16)
        wv_s = pa("wv_s", [128, 2, 256], BF16)
        wm_r = pa("wm_r", [64, 4, 256], BF16)   # head-major merge weights
        m1_s = pa("m1_s", [128, 4, 512], BF16)
        m2_s = pa("m2_s", [128, 4, 256], BF16)
        d1_s = pa("d1_s", [128, 256], BF16)
        d2_s = pa("d2_s", [128, 2, 256], BF16)
        # row-scaled weight copies (standardize folded in); single-buffered,
        # rescaled for the other graph between rounds. Bias corrections use
        # the ORIGINAL weights with rhs (inv*m), so they don't depend on
        # these copies.
        wq_c = pa("wq_c", [128, 2, 256], BF16)
        wk_c = pa("wk_c", [128, 2, 256], BF16)
        wv_c = pa("wv_c", [128, 2, 256], BF16)
        m1_c = pa("m1_c", [128, 2, 512], BF16)
        d1_c = {g: pa(f"d1_c{g}", [128, 256], BF16) for g in "st"}
        bq_c = {g: pa(f"bq_c{g}", [128, 2, 1], FP32) for g in "st"}
        bm_c = {g: pa(f"bm_c{g}", [128, 2, 1], FP32) for g in "st"}
        imv_bf = {g: pa(f"imv_bf{g}", [128, 2, 1], BF16) for g in "st"}
        s1b = {g: pa(f"s1b_{g}", [128, 2, 1], BF16) for g in "st"}
        s1h = {g: pa(f"s1h_{g}", [64, 4, 1], BF16) for g in "st"}
        rc4 = pa("rc4", [65, 4, 512], BF16)     # per-head softmax denominators
        ones_bf = pa("ones_bf", [65, 64], BF16)
        bq_s = pa("bq_s", [128, 2, 1], FP32)
        bm_s = pa("bm_s", [128, 2, 1], FP32)
        m1g_s = pa("m1g_s", [128, 4, 1], FP32)
        m1be_s = pa("m1be_s", [128, 4, 1], FP32)
        d1g_s = pa("d1g_s", [128, 2, 1], FP32)
        d1be_s = pa("d1be_s", [128, 2, 1], FP32)
        d2b_s = pa("d2b_s", [128, 2, 1], FP32)
        pstat = pa("pstat", [128, 2, 8], FP32)  # own-shard stat partials
        tstat = pa("tstat", [128, 2, 8], FP32)  # AllReduced stats
        red8 = pa("red8", [128, 2, 8], FP32)    # x1 stat scratch
        stx = pa("stx", [128, 2, 2], FP32)
        parf = pa("parf", [128, 4, 4], FP32)    # m1 stats partials (s:0-1 t:2-3)
        rsum = pa("rsum", [128, 4, 2], FP32)    # relu col sums (s, t)
        totf = pa("totf", [128, 4, 4], FP32)
        tsum = pa("tsum", [128, 4, 2], FP32)
        sdif = pa("sdif", [128, 4, 1], BF16)
        dlt = pa("dlt", [128, 2, 1], FP32)
        dsq = pa("dsq", [128, 2, 1], FP32)
        res = pa("res", [1, 1], FP32)
        chn = pa("chn", [1, 1], FP32)
        sv = pa("sv", [128, 160], FP32)   # scalar scratch, allocator below

        _svc = [0]

        def scol(n=1):
            b = _svc[0]
            _svc[0] += n
            assert _svc[0] <= 160
            return [sv[:, b + i:b + i + 1] for i in range(n)]

        dma = nc.sync.dma_start

        def coll(kind, op, in_t, out_t):
            nc.gpsimd.collective_compute(kind, op, replica_groups=RG,
                                         ins=[in_t.opt()],
                                         outs=[out_t.opt()])

        def inv_std(var_ap, eps):
            # rsqrt entirely on DVE (magic seed + 3 Newton steps)
            v, y, a = scol(3)
            nc.vector.tensor_scalar_add(v, var_ap, float(eps))
            nc.vector.tensor_scalar(y.bitcast(I32), v.bitcast(I32), 1, None,
                                    op0=ALU.logical_shift_right)
            nc.vector.tensor_scalar(y.bitcast(I32), y.bitcast(I32), -1,
                                    0x5F3759DF, op0=ALU.mult, op1=ALU.add)
            for _ in range(3):
                nc.vector.tensor_mul(a, y, y)
                nc.vector.tensor_mul(a, a, v)
                nc.vector.tensor_scalar(a, a, -0.5, 1.5,
                                        op0=ALU.mult, op1=ALU.add)
                nc.vector.tensor_mul(y, y, a)
            return y

        # ---------------- own-shard loads + partial stats ----------------
        dma(os_raw[:, :, :],
            xin[:, SOPS:SOPS + 1024].rearrange("p (g f) -> p g f", g=2))
        dma(ot_raw[:, :, :],
            xin[:, SOPT:SOPT + 1024].rearrange("p (g f) -> p g f", g=2))
        dma(ods_r[:, :], xin[:, SODS:SODS + 512])
        dma(odt_r[:, :], xin[:, SODT:SODT + 512])
        dma(chn[:, :], chain[:, :])
        nc.gpsimd.memset(pstat[:, :, :], 0.0)
        nc.gpsimd.memset(ones[:, :], 1.0)
        nc.gpsimd.memset(ones_bf[:, :], 1.0)
        for gi, oraw in ((0, os_raw), (1, ot_raw)):
            sqb = Prd.tile([128, 2, 512], BF16, tag="rdc", name=f"sqb{gi}")
            nc.vector.reduce_sum(pstat[:, :, 2 * gi], oraw[:, :, :], axis=AX.X)
            nc.vector.tensor_mul(sqb[:, :, :], oraw[:, :, :], oraw[:, :, :])
            nc.vector.reduce_sum(pstat[:, :, 2 * gi + 1], sqb[:, :, :],
                                 axis=AX.X)
        for di, odr in ((0, ods_r), (1, odt_r)):
            sqd = Prd.tile([128, 2, 512], BF16, tag="rdc", name=f"sqd{di}")
            nc.vector.reduce_sum(pstat[:, 0, 4 + 2 * di:4 + 2 * di + 1],
                                 odr[:, :].rearrange("p (o f) -> p o f", o=1),
                                 axis=AX.X)
            nc.vector.tensor_mul(sqd[:, 0, :], odr[:, :], odr[:, :])
            nc.vector.reduce_sum(pstat[:, 0, 5 + 2 * di:5 + 2 * di + 1],
                                 sqd[:, 0:1, :], axis=AX.X)
        ar0_in = Dram.tile([2 * 128, 8], FP32, name="ar0_in")
        ar0_out = Dram.tile([2 * 128, 8], FP32, name="ar0_out",
                            addr_space="Shared")
        for cg in range(2):
            dma(ar0_in[cg * 128:(cg + 1) * 128, :], pstat[:, cg, :])
        coll("AllReduce", ALU.add, ar0_in, ar0_out)
        for cg in range(2):
            dma(tstat[:, cg, :], ar0_out[cg * 128:(cg + 1) * 128, :])

        # ---------------- input AllGather + loads ----------------
        agi = Dram.tile([128, SHW], BF16, name="agi")
        ago = Dram.tile([NC * 128, SHW], BF16, name="ago",
                        addr_space="Shared")
        dma(agi[:, :], xin[:, :])
        coll("AllGather", ALU.bypass, agi, ago)
        for r in range(NC):
            rs = slice(r * 128, (r + 1) * 128)
            for g in range(2):
                dma(xs_bf[:, g, r, :], ago[rs, SOPS + g * 512:
                                           SOPS + (g + 1) * 512])
                dma(xt_bf[:, g, r, :], ago[rs, SOPT + g * 512:
                                           SOPT + (g + 1) * 512])
            dma(dsr[:, r, :], ago[rs, SODS:SODS + 512])
            dma(dtr[:, r, :], ago[rs, SODT:SODT + 512])

        def wld(dst, nm, gcols, ngroups, flat=False):
            base = WOFF[nm]
            for g in range(ngroups):
                lo, hi = base + g * gcols, base + (g + 1) * gcols
                a = lo
                while a < hi:
                    ch = a // WCH
                    b = min(hi, (ch + 1) * WCH)
                    src = ago[ch * 128:(ch + 1) * 128,
                              SW + a - ch * WCH:SW + b - ch * WCH]
                    if flat:
                        dma(dst[:, a - lo:b - lo], src)
                    else:
                        dma(dst[:, g, a - lo:b - lo], src)
                    a = b

        wld(wq_s, "wq", 256, 2)
        wld(wk_s, "wk", 256, 2)
        wld(wv_s, "wv", 256, 2)
        wld(m1_s, "m1", 512, 4)
        wld(m2_s, "m2", 256, 4)
        wld(d1_s, "d1", 256, 1, flat=True)
        wld(d2_s, "d2", 256, 2)
        for h in range(H):
            ch = (WOFF["wm"] + (h // 2) * 256) // WCH
            co = WOFF["wm"] + (h // 2) * 256 - ch * WCH
            dma(wm_r[:, h, :],
                ago[ch * 128 + (h % 2) * 64:ch * 128 + (h % 2) * 64 + 64,
                    SW + co:SW + co + 256])

        def fld(dst, nm, g):
            o = _FLAY[nm]
            dma(dst[:, :, :], fin[:, o:o + g].rearrange("p (g c) -> p g c", c=1))

        fld(bq_s, "bq", 2)
        fld(bm_s, "bm", 2)
        fld(m1g_s, "m1g", 4)
        fld(m1be_s, "m1be", 4)
        fld(d1g_s, "d1g", 2)
        fld(d1be_s, "d1be", 2)
        fld(d2b_s, "d2b", 2)

        # ---- derive inv/mean from AllReduced stats; fold into weights ----
        invp, nbp = {}, {}
        for gi, g in ((0, "s"), (1, "t")):
            for cg in range(2):
                mu, va = scol(2)
                nc.vector.tensor_scalar_mul(
                    mu, tstat[:, cg, 2 * gi:2 * gi + 1], 1.0 / N)
                nc.vector.tensor_scalar_mul(
                    va, tstat[:, cg, 2 * gi + 1:2 * gi + 2], 1.0 / N)
                msq, = scol(1)
                nc.vector.tensor_mul(msq, mu, mu)
                nc.vector.tensor_sub(va, va, msq)
                inv = inv_std(va, 0.0)
                (nb,) = scol(1)
                nc.vector.tensor_mul(nb, mu, inv)
                nc.vector.tensor_scalar_mul(nb, nb, -1.0)
                invp[(g, cg)], nbp[(g, cg)] = inv, nb
                # imv = inv*m = -nb (bias-correction matvec rhs)
                nc.vector.tensor_scalar_mul(imv_bf[g][:, cg, :], nb, -1.0)
            # bias corrections via ORIGINAL weights: Wq'^T m == Wq^T (inv*m)
            qsh = Psc.tile([128, 2, 512], FP32, tag="sc", name=f"qsh_{g}")
            for og in range(2):
                for cg in range(2):
                    nc.tensor.matmul(qsh[:, og, 0:1],
                                     wq_s[:, cg, og * 128:(og + 1) * 128],
                                     imv_bf[g][:, cg, :],
                                     start=(cg == 0), stop=(cg == 1))
            for og in range(2):
                nc.vector.tensor_sub(bq_c[g][:, og, :], bq_s[:, og, :],
                                     qsh[:, og, 0:1])
            vsh = Psc.tile([128, 2, 512], FP32, tag="sc", name=f"vsh_{g}")
            for og in range(2):
                for cg in range(2):
                    nc.tensor.matmul(vsh[:, og, 0:1],
                                     wv_s[:, cg, og * 128:(og + 1) * 128],
                                     imv_bf[g][:, cg, :],
                                     start=(cg == 0), stop=(cg == 1))
            for og in range(2):
                nc.vector.tensor_copy(s1b[g][:, og, :], vsh[:, og, 0:1])
            for h in range(H):
                dma(s1h[g][:, h, :],
                    s1b[g][(h % 2) * 64:(h % 2) * 64 + 64, h // 2, :])
            msh = Psc.tile([128, 2, 512], FP32, tag="sc", name=f"msh_{g}")
            for og in range(2):
                for h in range(H):
                    nc.tensor.matmul(
                        msh[:, og, 0:1], wm_r[:, h, og * 128:(og + 1) * 128],
                        s1h[g][:, h, :],
                        start=(h == 0), stop=(h == 3))
            for og in range(2):
                nc.vector.tensor_sub(bm_c[g][:, og, :], bm_s[:, og, :],
                                     msh[:, og, 0:1])

        def scale_w(dst, src, gkey, half=False):
            for cg in range(2):
                nc.vector.tensor_scalar_mul(dst[:, cg, :], src[:, cg, :],
                                            invp[(gkey, cg)])

        # r1a needs Wq'(s), Wk'(t), Wv'(t)
        scale_w(wq_c, wq_s, "s")
        scale_w(wk_c, wk_s, "t")
        scale_w(wv_c, wv_s, "t")
        scale_w(m1_c, m1_s, "s")
        # dis: inv only (shift absorbed by BN after d1)
        for di, g in ((0, "s"), (1, "t")):
            mu, va, msq = scol(3)
            nc.vector.tensor_scalar_mul(
                mu, tstat[:, 0, 4 + 2 * di:5 + 2 * di], 1.0 / N)
            nc.vector.tensor_scalar_mul(
                va, tstat[:, 0, 5 + 2 * di:6 + 2 * di], 1.0 / N)
            nc.vector.tensor_mul(msq, mu, mu)
            nc.vector.tensor_sub(va, va, msq)
            inv = inv_std(va, 0.0)
            nc.vector.tensor_scalar_mul(d1_c[g][:, :], d1_s[:, :], inv)

        # ---------------- attention tensors ----------------
        VT = pb("VT", [128, MT, H * HB], BF16)
        Kb = pb("Kb", [128, 2, 8, 512], BF16)
        ds_full = pb("ds_full", [128, 2, 8, 512], BF16)
        dt_full = pb("dt_full", [128, 2, 8, 512], BF16)
        d0f = pa("d0f", [128, 2, 8, 512], BF16, tag="xs_bf")
        for h in range(H):
            nc.gpsimd.memset(VT[:, :, h * HB + HD], 1.0)

        def attention(tag, q_own, k_src, v_src, out_bf, wq, bq, wk, wv, bm):
            # Q projection (+bias)
            qp = Psc.tile([128, 2, 512], FP32, tag="sc", name=f"qp_{tag}")
            for og in range(2):
                for cg in range(2):
                    nc.tensor.matmul(qp[:, og, :],
                                     wq[:, cg, og * 128:(og + 1) * 128],
                                     q_own[:, cg, :],
                                     start=(cg == 0), stop=(cg == 1))
            for og in range(2):
                nc.vector.tensor_scalar_add(Qb[:, og, :], qp[:, og, :],
                                            bq[:, og, :])
            # K projection, full N (no bias: softmax-invariant)
            for og in range(2):
                for c in range(8):
                    kp = Psc.tile([128, 2, 512], FP32, tag="sc", name=f"kp_{tag}")
                    for cg in range(2):
                        nc.tensor.matmul(kp[:, 0, :],
                                         wk[:, cg, og * 128:(og + 1) * 128],
                                         k_src[:, cg, c, :],
                                         start=(cg == 0), stop=(cg == 1))
                    nc.vector.tensor_copy(Kb[:, og, c, :], kp[:, 0, :])
            # V^T projection (keys on partitions), bias folded into bm
            for m in range(MT):
                c, f0 = divmod(m * 128, 512)
                vp = Psc.tile([128, 2, 512], FP32, tag="sc", name=f"vp_{tag}")
                for cg in range(2):
                    nc.tensor.matmul(vp[:, 0, 0:256],
                                     v_src[:, cg, c, f0:f0 + 128],
                                     wv[:, cg, :],
                                     start=(cg == 0), stop=(cg == 1))
                nc.vector.tensor_copy(
                    VT[:, m, :].rearrange("p (h c) -> p h c", c=HB)[:, :, 0:HD],
                    vp[:, 0, 0:256].rearrange("p (h c) -> p h c", c=HD))
            # streaming attention per head, PV pipelined one group behind
            for h in range(H):
                hg, hp = h // 2, (h % 2) * 64
                op = Pout.tile([65, 512], FP32, tag="out", name=f"op_{tag}{h}")
                prev = None
                for g in range(16):
                    sc = Psc.tile([128, 2, 512], FP32, tag="sc", name=f"sc_{tag}")
                    for j in range(2):
                        m = g * 2 + j
                        c, f0 = divmod(m * 128, 512)
                        nc.tensor.matmul(sc[:, j, :],
                                         Kb[hp:hp + 64, hg, c, f0:f0 + 128],
                                         Qb[hp:hp + 64, hg, :],
                                         start=True, stop=True)
                    pr = Ppr.tile([128, 2, 512], BF16, tag="pr", name=f"pr_{tag}",
                                  bufs=2)
                    nc.scalar.activation(pr[:, :, :], sc[:, :, :], AF.Exp,
                                         scale=0.125)
                    if prev is not None:
                        for j in range(2):
                            m = prev[0] * 2 + j
                            nc.tensor.matmul(
                                op[:, :], VT[:, m, h * HB:(h + 1) * HB],
                                prev[1][:, j, :], start=(m == 0), stop=False)
                    prev = (g, pr)
                for j in range(2):
                    m = prev[0] * 2 + j
                    nc.tensor.matmul(op[:, :], VT[:, m, h * HB:(h + 1) * HB],
                                     prev[1][:, j, :], start=False,
                                     stop=(m == MT - 1))
                # evacuate raw numerator (bf16); denominator reciprocal runs
                # on DVE straight out of PSUM partition 64; the PE broadcast
                # + normalize are deferred past the head loop
                nc.vector.tensor_copy(An[:, h, :], op[0:64, :])
                with nc.allow_low_precision(
                        reason="softmax denom as bf16: per-query scale "
                               "noise averages out in the final means"):
                    nc.vector.reciprocal(rc4[64:65, h, :], op[64:65, :])
            for h in range(H):
                bc = Pbc.tile([64, 512], FP32, tag="bc", name=f"bc_{tag}{h}")
                nc.tensor.matmul(bc[:, :], ones_bf[64:65, 0:64],
                                 rc4[64:65, h, :], start=True, stop=True)
                nc.vector.tensor_mul(An[:, h, :], An[:, h, :], bc[:, :])
            # merge: accumulate per head (K=64); bias only when not absorbed
            mg = Psc.tile([128, 2, 512], FP32, tag="sc", name=f"mg_{tag}")
            for og in range(2):
                for h in range(H):
                    nc.tensor.matmul(mg[:, og, :],
                                     wm_r[:, h, og * 128:(og + 1) * 128],
                                     An[:, h, :],
                                     start=(h == 0), stop=(h == 3))
            for og in range(2):
                if bm is not None:
                    nc.vector.tensor_scalar_add(out_bf[:, og, :], mg[:, og, :],
                                                bm[:, og, :])
                else:
                    nc.vector.tensor_copy(out_bf[:, og, :], mg[:, og, :])

        def gate_and_send(delta, oraw, gkey, agin):
            # g = delta * std(own raw);  std = inv*raw + nb  per cg group
            for cg in range(2):
                nc.scalar.activation(xsd[:, cg, :], oraw[:, cg, :],
                                     AF.Identity, bias=nbp[(gkey, cg)],
                                     scale=invp[(gkey, cg)])
                nc.vector.tensor_mul(g_bf[:, cg, :], delta[:, cg, :],
                                     xsd[:, cg, :])
                dma(agin[cg * 128:(cg + 1) * 128, :], g_bf[:, cg, :])

        # ---- mlp_dis pieces (replicated full N + own shard) ----
        x1fl = {"s": pb("x1f_s", [128, 2, 8, 512], BF16, tag="x1f"),
                "t": pb("x1f_t", [128, 2, 8, 512], BF16, tag="xt_bf")}
        x1og = {"s": pb("x1o_s", [128, 2, 512], BF16),
                "t": pb("x1o_t", [128, 2, 512], BF16)}
        rdo = pb("rdo", [128, 2, 512], BF16)
        dab = {}

        def dis_d1(g, draw, odr):
            for og in range(2):
                for c in range(8):
                    mp = Psc.tile([128, 2, 512], FP32, tag="sc", name="mp_d1")
                    nc.tensor.matmul(mp[:, 0, :],
                                     d1_c[g][:, og * 128:(og + 1) * 128],
                                     draw[:, c, :], start=True, stop=True)
                    nc.vector.tensor_copy(x1fl[g][:, og, c, :], mp[:, 0, :])
                mpo = Psc.tile([128, 2, 512], FP32, tag="sc", name="mp_d1o")
                nc.tensor.matmul(mpo[:, 0, :],
                                 d1_c[g][:, og * 128:(og + 1) * 128],
                                 odr[:, :], start=True, stop=True)
                nc.vector.tensor_copy(x1og[g][:, og, :], mpo[:, 0, :])

        def dis_stats(g):
            x1 = x1fl[g]
            nc.vector.reduce_sum(red8[:, :, :], x1[:, :, :, :], axis=AX.X)
            nc.vector.reduce_sum(stx[:, :, 0], red8[:, :, :], axis=AX.X)
            for c in range(8):
                sqc = Prd.tile([128, 2, 512], BF16, tag="rdc",
                               name=f"sqc_{g}{c}")
                nc.vector.tensor_mul(sqc[:, :, :], x1[:, :, c, :],
                                     x1[:, :, c, :])
                nc.vector.reduce_sum(red8[:, :, c], sqc[:, :, :], axis=AX.X)
            nc.vector.reduce_sum(stx[:, :, 1], red8[:, :, :], axis=AX.X)
            ab = []
            for og in range(2):
                mu, va, msq = scol(3)
                nc.vector.tensor_scalar_mul(mu, stx[:, og, 0:1], 1.0 / N)
                nc.vector.tensor_scalar_mul(va, stx[:, og, 1:2], 1.0 / N)
                nc.vector.tensor_mul(msq, mu, mu)
                nc.vector.tensor_sub(va, va, msq)
                inv = inv_std(va, EPS)
                a_, b_ = scol(2)
                nc.vector.tensor_mul(a_, d1g_s[:, og, :], inv)
                nc.vector.tensor_mul(b_, mu, a_)
                nc.vector.tensor_scalar_mul(b_, b_, -1.0)
                nc.vector.tensor_add(b_, b_, d1be_s[:, og, :])
                ab.append((a_, b_))
            dab[g] = ab

        def dis_d2(g, dfull, downb):
            ab = dab[g]
            for og in range(2):
                nc.scalar.activation(rdo[:, og, :], x1og[g][:, og, :],
                                     AF.Relu, bias=ab[og][1], scale=ab[og][0])
            for og in range(2):
                mpo = Psc.tile([128, 2, 512], FP32, tag="sc", name="mp_d2o")
                for cg in range(2):
                    nc.tensor.matmul(mpo[:, 0, :],
                                     d2_s[:, cg, og * 128:(og + 1) * 128],
                                     rdo[:, cg, :], start=(cg == 0),
                                     stop=(cg == 1))
                nc.vector.tensor_scalar_add(downb[:, og, :], mpo[:, 0, :],
                                            d2b_s[:, og, :])
            for c in range(8):
                rdc = Prd.tile([128, 2, 512], BF16, tag="rdc",
                               name=f"rdc_{g}{c}")
                for cg in range(2):
                    nc.scalar.activation(rdc[:, cg, :],
                                         x1fl[g][:, cg, c, :],
                                         AF.Relu, bias=ab[cg][1],
                                         scale=ab[cg][0])
                for og in range(2):
                    mp = Psc.tile([128, 2, 512], FP32, tag="sc",
                                  name="mp_d2")
                    for cg in range(2):
                        nc.tensor.matmul(
                            mp[:, 0, :],
                            d2_s[:, cg, og * 128:(og + 1) * 128],
                            rdc[:, cg, :],
                            start=(cg == 0), stop=(cg == 1))
                    nc.vector.tensor_scalar_add(dfull[:, og, c, :],
                                                mp[:, 0, :],
                                                d2b_s[:, og, :])

        # ---------------- main sequence ----------------
        ag0_in = Dram.tile([D, NQ], BF16, name="ag0_in")
        ag0_out = Dram.tile([NC * D, NQ], BF16, name="ag0_out",
                            addr_space="Shared")
        ag1_in = Dram.tile([D, NQ], BF16, name="ag1_in")
        ag1_out = Dram.tile([NC * D, NQ], BF16, name="ag1_out",
                            addr_space="Shared")

        dis_d1("s", dsr, ods_r)
        attention("r1a", os_raw, xt_bf, xt_bf, d_bf,
                  wq_c, bq_c["s"], wk_c, wv_c, bm_c["t"])
        mkdbg(dma, "dbg_dbf", d_bf[:, :, :], (128, 2, 512), BF16)
        gate_and_send(d_bf, ot_raw, "t", ag0_in)
        coll("AllGather", ALU.bypass, ag0_in, ag0_out)
        # rescale projection weights for r1b: Wq'(t), Wk'(s), Wv'(s)
        scale_w(wq_c, wq_s, "t")
        scale_w(wk_c, wk_s, "s")
        scale_w(wv_c, wv_s, "s")

        dis_d1("t", dtr, odt_r)
        dis_stats("s")
        dis_stats("t")
        dis_d2("s", ds_full, ds_own_bf)
        dis_d2("t", dt_full, dt_own_bf)
        mkdbg(dma, "dbg_dsown", ds_own_bf[:, :, :], (128, 2, 512), BF16)
        mkdbg(dma, "dbg_dsfull", ds_full[:, :, :, :], (128, 2, 8, 512), BF16)

        # ---------------- round 1b ----------------
        attention("r1b", ot_raw, xs_bf, xs_bf, d_bf,
                  wq_c, bq_c["t"], wk_c, wv_c, bm_c["s"])
        gate_and_send(d_bf, os_raw, "s", ag1_in)
        coll("AllGather", ALU.bypass, ag1_in, ag1_out)

        # gather gated delta0 (values for r2a)
        for r in range(NC):
            for g in range(2):
                dma(d0f[:, g, r, :],
                    ag0_out[r * D + g * 128:r * D + (g + 1) * 128, :])
        mkdbg(dma, "dbg_d0f", d0f[:, :, :, :], (128, 2, 8, 512), BF16)

        # ---------------- round 2a + m1_s ----------------
        attention("r2a", ds_own_bf, dt_full, d0f, dp0_bf,
                  wq_s, bq_s, wk_s, wv_s, None)
        mkdbg(dma, "dbg_dp0", dp0_bf[:, :, :], (128, 2, 512), BF16)

        x1_s = pb("x1_s", [128, 4, 512], BF16, tag="dsr")
        x1_t = pb("x1_t", [128, 4, 512], BF16, tag="Kb")
        sq = pb("sq", [128, 4, 512], BF16, tag="x1f")

        def m1_block(x1, oraw, dp, gkey, pcol):
            rhs = [oraw[:, 0, :], oraw[:, 1, :], dp[:, 0, :], dp[:, 1, :]]
            for og in range(4):
                mp = Psc.tile([128, 2, 512], FP32, tag="sc", name=f"mp_m1{gkey}")
                for cg in range(4):
                    lhs = (m1_c[:, cg, og * 128:(og + 1) * 128]
                           if cg < 2 else
                           m1_s[:, cg, og * 128:(og + 1) * 128])
                    nc.tensor.matmul(mp[:, 0, :], lhs, rhs[cg],
                                     start=(cg == 0), stop=(cg == 3))
                nc.vector.tensor_copy(x1[:, og, :], mp[:, 0, :])
            nc.vector.reduce_sum(parf[:, :, pcol], x1[:, :, :], axis=AX.X)
            nc.vector.tensor_mul(sq[:, :, :], x1[:, :, :], x1[:, :, :])
            nc.vector.reduce_sum(parf[:, :, pcol + 1], sq[:, :, :], axis=AX.X)

        m1_block(x1_s, os_raw, dp0_bf, "s", 0)
        scale_w(m1_c, m1_s, "t")
        ars_in = Dram.tile([2 * D, 2], FP32, name="ars_in")
        ars_out = Dram.tile([2 * D, 2], FP32, name="ars_out",
                            addr_space="Shared")
        for g in range(4):
            dma(ars_in[g * 128:(g + 1) * 128, :], parf[:, g, 0:2])
        coll("AllReduce", ALU.add, ars_in, ars_out)

        # gather gated delta1 (values for r2b) into the xt_bf slot
        pt_f = pa("pt_f", [128, 2, 8, 512], BF16, tag="xt_bf")
        for r in range(NC):
            for g in range(2):
                dma(pt_f[:, g, r, :],
                    ag1_out[r * D + g * 128:r * D + (g + 1) * 128, :])

        # ---------------- round 2b + m1_t + final ----------------
        attention("r2b", dt_own_bf, ds_full, pt_f, dp1_bf,
                  wq_s, bq_s, wk_s, wv_s, None)
        mkdbg(dma, "dbg_dp1", dp1_bf[:, :, :], (128, 2, 512), BF16)

        r_f = pb("r_f", [128, 4, 512], BF16, tag="dtr")
        for g in range(4):
            dma(totf[:, g, 0:2], ars_out[g * 128:(g + 1) * 128, :])

        def bn_apply_relu_sum(x1, c0, sum_col):
            for og in range(4):
                mu, va, msq, a_, b_ = scol(5)
                nc.vector.tensor_scalar_mul(mu, totf[:, og, c0:c0 + 1],
                                            1.0 / N)
                nc.vector.tensor_scalar_mul(va, totf[:, og, c0 + 1:c0 + 2],
                                            1.0 / N)
                nc.vector.tensor_mul(msq, mu, mu)
                nc.vector.tensor_sub(va, va, msq)
                inv = inv_std(va, EPS)
                nc.vector.tensor_mul(a_, m1g_s[:, og, :], inv)
                nc.vector.tensor_mul(b_, mu, a_)
                nc.vector.tensor_scalar_mul(b_, b_, -1.0)
                nc.vector.tensor_add(b_, b_, m1be_s[:, og, :])
                nc.scalar.activation(r_f[:, og, :], x1[:, og, :], AF.Relu,
                                     bias=b_, scale=a_)
            nc.vector.reduce_sum(rsum[:, :, sum_col], r_f[:, :, :], axis=AX.X)

        bn_apply_relu_sum(x1_s, 0, 0)
        m1_block(x1_t, ot_raw, dp1_bf, "t", 2)

        # AllReduce #2: [stats_t (2 cols) | sums_s (1 col)]
        ar2_in = Dram.tile([2 * D, 3], FP32, name="ar2_in")
        ar2_out = Dram.tile([2 * D, 3], FP32, name="ar2_out",
                            addr_space="Shared")
        for g in range(4):
            dma(ar2_in[g * 128:(g + 1) * 128, 0:2], parf[:, g, 2:4])
            dma(ar2_in[g * 128:(g + 1) * 128, 2:3], rsum[:, g, 0:1])
        coll("AllReduce", ALU.add, ar2_in, ar2_out)
        for g in range(4):
            dma(totf[:, g, 2:4], ar2_out[g * 128:(g + 1) * 128, 0:2])
            dma(tsum[:, g, 0:1], ar2_out[g * 128:(g + 1) * 128, 2:3])

        bn_apply_relu_sum(x1_t, 2, 1)

        # AllReduce #3: sums_t
        ar3_in = Dram.tile([2 * D, 1], FP32, name="ar3_in")
        ar3_out = Dram.tile([2 * D, 1], FP32, name="ar3_out",
                            addr_space="Shared")
        for g in range(4):
            dma(ar3_in[g * 128:(g + 1) * 128, :], rsum[:, g, 1:2])
        coll("AllReduce", ALU.add, ar3_in, ar3_out)
        for g in range(4):
            dma(tsum[:, g, 1:2], ar3_out[g * 128:(g + 1) * 128, :])

        # delta = m2 @ (S_s - S_t)/N ; result = ||delta||^2
        for g in range(4):
            df, = scol(1)
            nc.vector.tensor_sub(df, tsum[:, g, 0:1], tsum[:, g, 1:2])
            nc.vector.tensor_scalar_mul(df, df, 1.0 / N)
            nc.vector.tensor_copy(sdif[:, g, :], df)
        mpv = Psc.tile([128, 2, 512], FP32, tag="sc", name="mpv")
        for og in range(2):
            for cg in range(4):
                nc.tensor.matmul(mpv[:, og, 0:1],
                                 m2_s[:, cg, og * 128:(og + 1) * 128],
                                 sdif[:, cg, :],
                                 start=(cg == 0), stop=(cg == 3))
        for og in range(2):
            nc.vector.tensor_copy(dlt[:, og, :], mpv[:, og, 0:1])
        nc.vector.tensor_mul(dsq[:, :, :], dlt[:, :, :], dlt[:, :, :])
        dot = Pbc.tile([64, 512], FP32, tag="bc", name="dot")
        for g in range(2):
            nc.tensor.matmul(dot[0:1, 0:1], dsq[:, g, :], ones[:, 0:1],
                             start=(g == 0), stop=(g == 1))
        nc.vector.tensor_copy(res[:, :], dot[0:1, 0:1])
        dma(out_dram[:, :], res[:, :])

        st.close()

    nc.compile()
    return nc


# head permutation: new row i = h*64+d  <- old channel d*4+h
_PERM = np.array([d * H + h for h in range(H) for d in range(HD)])


def _prep_inputs(inputs):
    bf16 = ml_dtypes.bfloat16
    f32 = np.float32

    def C(x, dt=f32):
        return np.ascontiguousarray(np.asarray(x), dtype=dt)

    p_src = C(inputs["p_src"])[0]
    p_tar = C(inputs["p_tar"])[0]
    dis_src = C(inputs["dis_src"])[0]
    dis_tar = C(inputs["dis_tar"])[0]
    aq_w = C(inputs["aq_w"]); ak_w = C(inputs["ak_w"])
    av_w = C(inputs["av_w"]); am_w = C(inputs["am_w"])

    wpad = np.zeros((128, WPAD), bf16)

    def putw(nm, arr):
        a = np.asarray(arr, bf16)
        R, Cc = a.shape
        o = WOFF[nm]
        for g in range(R // 128):
            wpad[:, o + g * Cc:o + (g + 1) * Cc] = a[g * 128:(g + 1) * 128, :]

    putw("wq", aq_w[_PERM, :].T)
    putw("wk", ak_w[_PERM, :].T)
    putw("wv", av_w[_PERM, :].T)
    putw("wm", am_w[:, _PERM].T)   # head-major rows -> 2 slabs (h01 | h23)
    putw("m1", C(inputs["m1_w"]).T)
    putw("m2", C(inputs["m2_w"]).T)
    putw("d1", C(inputs["d1_w"]).T)
    putw("d2", C(inputs["d2_w"]).T)

    fb = np.zeros((128, WF), f32)

    def putf(nm, vec):
        v = np.asarray(vec, f32).reshape(-1)
        o = _FLAY[nm]
        for g in range(v.size // 128):
            fb[:, o + g] = v[g * 128:(g + 1) * 128]

    putf("bq", C(inputs["aq_b"])[_PERM])
    putf("bm", C(inputs["am_b"]) + am_w @ C(inputs["av_b"]))
    putf("m1g", inputs["m1_g"])
    putf("m1be", inputs["m1_be"])
    putf("d1g", inputs["d1_g"])
    putf("d1be", inputs["d1_be"])
    putf("d2b", inputs["d2_b"])

    in_maps = []
    for c in range(NC):
        sl = slice(c * NQ, (c + 1) * NQ)
        x = np.zeros((128, SHW), bf16)
        for o, src in ((SOPS, p_src), (SOPT, p_tar)):
            own = np.asarray(src[sl, :].T, bf16)
            x[:, o:o + 512] = own[0:128]
            x[:, o + 512:o + 1024] = own[128:256]
        x[:, SODS:SODS + 512] = np.asarray(dis_src[sl, :].T, bf16)
        x[:, SODT:SODT + 512] = np.asarray(dis_tar[sl, :].T, bf16)
        x[:, SW:SHW] = wpad[:, c * WCH:(c + 1) * WCH]
        in_maps.append({"xin": x, "fin": fb,
                        "chain": np.zeros((1, 1), f32)})
    return in_maps


def kernel(**inputs):
    from concourse.bass_utils import run_bass_kernel_spmd

    if "nc" not in _CACHE:
        _CACHE["nc"] = _build_program()
    nc = _CACHE["nc"]
    in_maps = _prep_inputs(inputs)
    res = run_bass_kernel_spmd(nc, in_maps, core_ids=list(range(NC)))
    return np.asarray(res.results[0]["out"], np.float32).reshape(())
